# revision 1
# baseline (speedup 1.0000x reference)
"""Causal multi-head attention (B=4, S=2048, D=1024, H=16, RoPE) on 8 trn2 cores.

Sharding: core c -> batch c//2, head-half c%2 (8 heads / 512 dims per core).
Each core computes QKV projections for its head slice, RoPE, causal flash
attention, and a partial output projection with its Wo column slice; the host
sums the two partials per batch (the tensor-parallel all-reduce) and
transposes back.

Layout choices (everything arranged so no on-device transposes and no
partition-shifted engine ops are needed):
  - host feeds x^T [1024,2048] and pre-transposed weight slices
  - Q^T/K^T computed in [d,t] layout; W rows pre-permuted per head to
    [even(32)|odd(32)] blocks so RoPE is block-structured
  - RoPE pair-swap via SBUF->SBUF DMA (DMA may cross partitions)
  - scores computed transposed (S^T[k,q]) so softmax normalizer comes from a
    ones-column fused into the PV matmul; exp is a single ScalarE pass
  - causal mask added inside the S^T PSUM accumulation via an extra matmul
    (-1e9*I @ U-pattern)
  - all matmuls run as float32r (full PE rate at free dim >= 256)
"""

import numpy as np

import concourse.bass as bass
import concourse.bacc as bacc
import concourse.mybir as mybir
import concourse.tile as tile
from concourse.bass import ds, ts
from concourse.bass_utils import run_bass_kernel_spmd

F32 = mybir.dt.float32
F32R = mybir.dt.float32r

B, S, D, H, DK = 4, 2048, 1024, 16, 64
THETA = 10000.0
NH = 8  # heads per core
HD = NH * DK  # 512 head dims per core
P = 128
NEG = -1.0e9

# tuning knobs (timeline-sim swept)
CFG = {
    "wpool_tags": 1,   # 1: single W slot; 2: alternate tags (prefetch)
    "xpool_bufs": 3,
    "swpool_bufs": 2,
    "evac_engine": "vector",  # "vector" | "any"
    "startup_split": True,     # first W-pass x pieces on the ACT HWDGE queue
    "ptile_bufs": 4,
    "pv_bufs": 3,
    "psC_bufs": 1,
}


def r32(ap):
    return ap.bitcast(F32R)


def build_attention_nc(nrep=1):
    nc = bacc.Bacc("TRN2", target_bir_lowering=False, debug=False)

    xT = nc.dram_tensor("xT", [D, S], F32R, kind="ExternalInput")
    wqT = nc.dram_tensor("wqT", [D, HD], F32R, kind="ExternalInput")
    wkT = nc.dram_tensor("wkT", [D, HD], F32R, kind="ExternalInput")
    wvT = nc.dram_tensor("wvT", [D, HD], F32R, kind="ExternalInput")
    woT = nc.dram_tensor("woT", [HD, D], F32R, kind="ExternalInput")
    cosP = nc.dram_tensor("cosP", [P, S], F32, kind="ExternalInput")
    sinP = nc.dram_tensor("sinP", [P, S], F32, kind="ExternalInput")
    umask = nc.dram_tensor("umask", [2, P, 512], mybir.dt.bfloat16, kind="ExternalInput")
    negI = nc.dram_tensor("negI", [P, P], mybir.dt.bfloat16, kind="ExternalInput")
    onesc = nc.dram_tensor("onesc", [P, 128], F32R, kind="ExternalInput")
    outT = nc.dram_tensor("outT", [D, S], F32, kind="ExternalOutput")
    rscr = nc.dram_tensor("rscr", [NH * 4, 512], F32, kind="Internal")

    with tile.TileContext(nc) as tc:
        if nrep == 1:
            _attention_tile(
                tc, xT, wqT, wkT, wvT, woT, cosP, sinP, umask, negI, onesc, outT, rscr
            )
        else:
            with tc.For_i(0, nrep, 1):
                _attention_tile(
                    tc, xT, wqT, wkT, wvT, woT, cosP, sinP, umask, negI, onesc,
                    outT, rscr,
                )
    nc.compile()
    return nc


def _attention_tile(tc, xT, wqT, wkT, wvT, woT, cosP, sinP, umask, negI, onesc, outT, rscr):
    nc = tc.nc

    with tc.tile_pool(name="qkv", bufs=1) as qkv:
        # ---- persistent tiles ----
        QT = qkv.tile([P, 4, S], F32, tag="QT")  # [d'%128, d'//128, t]
        KT = qkv.tile([P, 4, S], F32, tag="KT")
        VP = qkv.tile([P, 16, 528], F32, tag="VP")  # [t%128, t//128, 8*(64+2)]

        # ones columns of V' (col 64 of each 66-wide head group)
        vp_g = VP[:, :, :].rearrange("p k (h c) -> p k h c", c=66)
        nc.sync.dma_start(
            r32(vp_g[:, :, :, 64:65]),
            onesc.ap().rearrange("p (k h one) -> p k h one", k=16, one=1),
        )

        xT_t = xT.ap().rearrange("(o p) t -> p o t", p=P)  # [128, 8, 2048]

        # ---- phase A: Q+K per shared x chunk, then V (x streamed 2x) ----
        with (
            tc.tile_pool(name="wpool", bufs=1) as wpool,
            tc.tile_pool(name="xpool", bufs=CFG["xpool_bufs"]) as xpool,
            tc.tile_pool(name="swpool", bufs=CFG["swpool_bufs"]) as swpool,
            tc.tile_pool(name="rconst", bufs=1) as rconst,
            tc.tile_pool(name="psA", bufs=6, space="PSUM") as psA,
        ):
            cos_sb = rconst.tile([P, S], F32, tag="cos")
            sin_sb = rconst.tile([P, S], F32, tag="sin")

            wq_sb = wpool.tile([P, 8, HD], F32R, tag="wq")
            wk_sb = wpool.tile([P, 8, HD], F32R, tag="wk")
            for i in range(8):
                nc.sync.dma_start(wq_sb[:, i, :], wqT.ap().rearrange("(o p) d -> p o d", p=P)[:, i, :])
            for i in range(8):
                nc.sync.dma_start(wk_sb[:, i, :], wkT.ap().rearrange("(o p) d -> p o d", p=P)[:, i, :])
            nc.sync.dma_start(cos_sb, cosP.ap())
            nc.sync.dma_start(sin_sb, sinP.ap())

            # pass 1: Q and K share each x chunk
            for tci in range(4):  # t chunks of 512
                xs = []
                for hb in range(2):
                    x_sb = xpool.tile([P, 4, 512], F32R, tag="x")
                    for q2 in range(2):
                        eng = (
                            nc.scalar
                            if CFG["startup_split"] and tci == 0
                            else nc.sync
                        )
                        eng.dma_start(
                            x_sb[:, ds(q2 * 2, 2), :],
                            xT_t[:, ds(hb * 4 + q2 * 2, 2), ds(tci * 512, 512)],
                        )
                    xs.append(x_sb)
                for dst, w_sb in ((QT, wq_sb), (KT, wk_sb)):
                    for j in range(4):  # d' tiles of 128
                        ps = psA.tile([P, 512], F32, tag="psA")
                        for i in range(8):
                            nc.tensor.matmul(
                                ps,
                                lhsT=r32(w_sb[:, i, ts(j, P)]),
                                rhs=r32(xs[i // 4][:, i % 4, :]),
                                start=(i == 0),
                                stop=(i == 7),
                            )
                        nc.any.tensor_copy(r32(dst[:, j, ds(tci * 512, 512)]), ps)

            # RoPE (overlaps the V pass below). Q/K interleaved per j so
            # head pair 0 unblocks attention early; K's multiplies run on the
            # otherwise-idle GPSIMD so both tensors rope in parallel.
            for j in range(4):
                for dst in (QT, KT):
                    mul_eng = nc.vector if dst is QT else nc.gpsimd
                    for hf in range(2):
                        hsl = ds(hf * 1024, 1024)
                        qsw = swpool.tile([P, 1024], F32, tag="qsw")
                        for blk in range(4):
                            sb = blk + (1 if blk % 2 == 0 else -1)
                            nc.scalar.dma_start(
                                qsw[blk * 32 : blk * 32 + 32, :],
                                dst[sb * 32 : sb * 32 + 32, j, hsl],
                            )
                        tmp = swpool.tile([P, 1024], F32, tag="rtmp")
                        mul_eng.tensor_mul(tmp, cos_sb[:, hsl], dst[:, j, hsl])
                        mul_eng.tensor_mul(qsw, sin_sb[:, hsl], qsw)
                        nc.vector.tensor_add(r32(dst[:, j, hsl]), tmp, qsw)

            # pass 2: V (x streamed again; wv reuses wq's slot space via tag)
            wv_sb = wpool.tile([P, 8, HD], F32R, tag="wq", name="wv")
            for i in range(8):
                nc.sync.dma_start(wv_sb[:, i, :], wvT.ap().rearrange("(o p) d -> p o d", p=P)[:, i, :])
            for tci in range(4):
                xs = []
                for hb in range(2):
                    x_sb = xpool.tile([P, 4, 512], F32R, tag="x")
                    for q2 in range(2):
                        nc.sync.dma_start(
                            x_sb[:, ds(q2 * 2, 2), :],
                            xT_t[:, ds(hb * 4 + q2 * 2, 2), ds(tci * 512, 512)],
                        )
                    xs.append(x_sb)
                for tt in range(4):  # t tiles of 128 within chunk
                    ps = psA.tile([P, 512], F32, tag="psA")
                    for i in range(8):
                        nc.tensor.matmul(
                            ps,
                            lhsT=r32(xs[i // 4][:, i % 4, ts(tt, P)]),
                            rhs=r32(wv_sb[:, i, :]),
                            start=(i == 0),
                            stop=(i == 7),
                        )
                    kt_idx = tci * 4 + tt
                    nc.any.tensor_copy(
                        r32(vp_g[:, kt_idx, :, 0:64]),
                        ps.rearrange("p (h c) -> p h c", c=64),
                    )

        # ---- phase B: attention per head (+ phase C overlapped) ----
        with (
            tc.tile_pool(name="mconst", bufs=1) as mconst,
            tc.tile_pool(name="ptile", bufs=CFG["ptile_bufs"]) as ptile,
            tc.tile_pool(name="srowp", bufs=2) as srowp,
            tc.tile_pool(name="scap", bufs=2) as scap,
            tc.tile_pool(name="evpool", bufs=3) as evpool,
            tc.tile_pool(name="wopool", bufs=1) as wopool,
            tc.tile_pool(name="obpool", bufs=3) as obpool,
            tc.tile_pool(name="psS", bufs=2, space="PSUM") as psS,
            tc.tile_pool(name="psPV", bufs=CFG["pv_bufs"], space="PSUM") as psPV,
            tc.tile_pool(name="psC", bufs=CFG["psC_bufs"], space="PSUM") as psC,
        ):
            HOP = mconst.tile([P, 4, S], F32, tag="HOP")  # head pairs x [128 dv, t]
            umask_sb = mconst.tile([P, 2, 512], mybir.dt.bfloat16, tag="umask")
            nc.sync.dma_start(umask_sb, umask.ap().rearrange("r p q -> p r q"))
            negI_sb = mconst.tile([P, P], mybir.dt.bfloat16, tag="negI")
            nc.sync.dma_start(negI_sb, negI.ap())

            # hoisted so the DMA can overlap attention
            wo_sb = wopool.tile([P, 4, D], F32R, tag="wo")
            wosrc = woT.ap().rearrange("(m p) o -> p m o", p=P)
            for i in range(4):
                nc.sync.dma_start(wo_sb[:, i, :], wosrc[:, i, :])
            outT_ap = outT.ap()
            for qb in range(4):  # q blocks of 512
                qsl = ds(qb * 512, 512)
                for m in range(4):  # head pairs; even rows 0:64, odd 64:128
                    pvs = []
                    for _e in range(2):
                        pv_t = psPV.tile([P, 512], F32, tag="pv", name=f"pv{_e}")
                        pvs.append(pv_t)
                    nkt = 4 * qb + 4
                    for kt in range(nkt):
                        roff = kt - 4 * qb
                        # live q cols start; floored at 256 so fp32r S/PV
                        # matmuls keep moving dim >= 256 (fast path)
                        c0 = min(128 * roff, 256) if roff >= 0 else 0
                        s2 = psS.tile([P, 2, 512], F32, tag="s")
                        first = [True, True]
                        if roff >= 0:
                            # causal mask (bf16: full rate at small N). The
                            # pattern zero-extends right, so cover the whole
                            # live region [c0:512] to keep PSUM groups
                            # region-consistent. roff=3 uses the [full|tri]
                            # pattern (its live region starts at the 256 floor)
                            mi = 1 if roff == 3 else 0
                            for e in range(2):
                                nc.tensor.matmul(
                                    s2[:, e, c0:],
                                    lhsT=negI_sb,
                                    rhs=umask_sb[:, mi, 0 : 512 - c0],
                                    start=True,
                                    stop=False,
                                )
                                first[e] = False
                        # paired S-MMs: disjoint PE row groups run concurrently
                        for e in range(2):
                            rb = e * 64
                            nc.tensor.matmul(
                                s2[:, e, c0:],
                                lhsT=r32(KT[rb : rb + 64, m, ts(kt, P)]),
                                rhs=r32(QT[rb : rb + 64, m, ds(qb * 512 + c0, 512 - c0)]),
                                start=first[e],
                                stop=True,
                            )
                        # one exp pass over both heads' live columns
                        pt2 = ptile.tile([P, 2, 512], F32R, tag="pt")
                        nc.scalar.activation(
                            pt2[:, :, c0:],
                            s2[:, :, c0:],
                            mybir.ActivationFunctionType.Exp,
                            scale=0.125,
                        )
                        for e in range(2):
                            nc.tensor.matmul(
                                pvs[e][0:65, c0:],
                                lhsT=r32(VP[:, kt, ds((2 * m + e) * 66, 65)]),
                                rhs=pt2[:, e, c0:],
                                start=(kt == 0),
                                stop=(kt == nkt - 1),
                            )
                    # normalization: recip the sum row, dump to DRAM, evac
                    # unscaled, then scale the pair tile in one TT
                    for e in range(2):
                        pv = pvs[e]
                        nc.vector.reciprocal(pv[64:65, :], pv[64:65, :])
                        srow = srowp.tile([65, 512], F32, tag="srow")
                        nc.vector.tensor_copy(srow[64:65, :], pv[64:65, :])
                        slot = (2 * m + e) * 4 + qb
                        nc.scalar.dma_start(
                            rscr.ap()[slot : slot + 1, :], srow[64:65, :]
                        )
                        cpeng = nc.vector if CFG["evac_engine"] == "vector" else nc.any
                        if e == 0:
                            cpeng.tensor_copy(r32(HOP[0:64, m, qsl]), pv[0:64, :])
                        else:
                            ev = evpool.tile([64, 512], F32R, tag="ev")
                            cpeng.tensor_copy(ev, pv[0:64, :])
                            nc.scalar.dma_start(r32(HOP[64:128, m, qsl]), ev)
                    sca = scap.tile([P, 512], F32, tag="sca")
                    for e in range(2):
                        slot = (2 * m + e) * 4 + qb
                        rsrc = bass.AP(
                            tensor=rscr.ap().tensor,
                            offset=slot * 512,
                            ap=[[0, 64], [1, 512]],
                        )
                        nc.scalar.dma_start(sca[e * 64 : e * 64 + 64, :], rsrc)
                    nc.vector.tensor_mul(
                        r32(HOP[:, m, qsl]), HOP[:, m, qsl], sca
                    )
                # phase C for this t-chunk (all pairs at this qb done)
                for ot in range(8):  # o tiles of 128
                    ps = psC.tile([P, 512], F32, tag="psC")
                    for mm in range(4):
                        nc.tensor.matmul(
                            ps,
                            lhsT=r32(wo_sb[:, mm, ts(ot, P)]),
                            rhs=r32(HOP[:, mm, qsl]),
                            start=(mm == 0),
                            stop=(mm == 3),
                        )
                    ob = obpool.tile([P, 512], F32, tag="ob")
                    (nc.vector if CFG["evac_engine"] == "vector" else nc.any).tensor_copy(ob, ps)
                    nc.sync.dma_start(outT_ap[ts(ot, P), qsl], ob)


# ---------------- host side ----------------

def _host_tables():
    i = np.arange(32, dtype=np.float32)
    inv_freq = (THETA ** (2.0 * i / DK)).astype(np.float32)
    t = np.arange(S, dtype=np.float32)
    ang = t[:, None] / inv_freq[None, :]  # [S, 32]
    c = np.cos(ang).astype(np.float32).T  # [32, S]
    sn = np.sin(ang).astype(np.float32).T
    cosP = np.tile(c, (4, 1))  # [128, S]
    sinP = np.tile(sn, (4, 1))
    sign = np.repeat(np.array([-1.0, 1.0, -1.0, 1.0], dtype=np.float32), 32)
    sinP = sinP * sign[:, None]

    import ml_dtypes

    kk = np.arange(P)[:, None]
    qq = np.arange(512)[None, :]
    umask = np.stack(
        [(kk > qq - 128 * r) for r in range(2)]
    ).astype(ml_dtypes.bfloat16)  # [2,128,512]: r0 tri, r1 [full|tri]
    negI = (NEG * np.eye(P)).astype(ml_dtypes.bfloat16)
    return cosP, sinP, umask, negI


_PERM = np.concatenate(
    [np.concatenate([h * 64 + np.arange(0, 64, 2), h * 64 + np.arange(1, 64, 2)])
     for h in range(NH)]
)

_NC_CACHE = {}


def make_in_maps(x, Wq, Wk, Wv, Wo):
    cosP, sinP, umask, negI = _host_tables()
    in_maps = []
    for c in range(8):
        b, hh = c // 2, c % 2
        sl = slice(hh * HD, (hh + 1) * HD)
        in_maps.append(
            {
                "xT": np.ascontiguousarray(x[b].T),
                "wqT": np.ascontiguousarray(Wq[sl, :][_PERM].T),
                "wkT": np.ascontiguousarray(Wk[sl, :][_PERM].T),
                "wvT": np.ascontiguousarray(Wv[sl, :].T),
                "woT": np.ascontiguousarray(Wo[:, sl].T),
                "cosP": cosP,
                "sinP": sinP,
                "umask": umask,
                "negI": negI,
                "onesc": np.ones((P, 128), dtype=np.float32),
            }
        )
    return in_maps


def gather_out(core_outs):
    out = np.empty((B, S, D), dtype=np.float32)
    for b in range(B):
        out[b] = (core_outs[2 * b]["outT"] + core_outs[2 * b + 1]["outT"]).T
    return out


def kernel(x, Wq, Wk, Wv, Wo):
    x = np.asarray(x, dtype=np.float32)
    Wq = np.asarray(Wq, dtype=np.float32)
    Wk = np.asarray(Wk, dtype=np.float32)
    Wv = np.asarray(Wv, dtype=np.float32)
    Wo = np.asarray(Wo, dtype=np.float32)

    if "nc" not in _NC_CACHE:
        _NC_CACHE["nc"] = build_attention_nc()
    nc = _NC_CACHE["nc"]

    in_maps = make_in_maps(x, Wq, Wk, Wv, Wo)
    res = run_bass_kernel_spmd(nc, in_maps, core_ids=list(range(8)))
    return gather_out(res.results)



# revision 3
# speedup vs baseline: 1.3155x; 1.3155x over previous
"""Causal multi-head attention (B=4, S=2048, D=1024, H=16, RoPE) on 8 trn2 cores.

Sharding: core c -> batch c//2, head-half c%2 (8 heads / 512 dims per core).
Each core computes QKV projections for its head slice, RoPE, causal flash
attention, and a partial output projection with its Wo column slice; the host
sums the two partials per batch (the tensor-parallel all-reduce) and
transposes back.

v2 layout/scheduling (vs the 406us baseline):
  - bf16 everywhere outside PSUM accumulation (inputs host-cast): same PE
    rate as fp32r but no 256-wide fp32r floor, half the DMA bytes, and 2x
    DVE throughput on the elementwise ops
  - x loaded ONCE into SBUF (32KB/partition) and reused for the V pass
  - pass 1 is j-outer so each (tensor, j) head-pair column finishes early;
    RoPE (swap-DMA + 3 DVE TTs) runs per (tensor, j) overlapped with the
    rest of pass 1 / the V pass; PSUM evacuations all on ACT (idle there)
  - causal mask matmuls cover only the 128-wide diagonal block (bf16 S
    matmuls don't need the fp32r >=256 moving width)
  - PV matmuls are issued one kt behind the S/exp pair so PE never waits
    on the exp latency; ACT carries exps only
  - softmax normalizer: ones-column fused in V' gives the row sum; DVE
    reciprocal -> DRAM round-trip broadcast on the Pool SWDGE queue (keeps
    the shared HWDGE free); the scale-multiply is deferred one m-block
  - phase C (output projection) groups are interleaved into the NEXT
    q-block's attention so the PSUM evac never stalls PE; the last qb's
    groups drain through spare psPV slots
"""

import numpy as np

import concourse.bass as bass
import concourse.bacc as bacc
import concourse.mybir as mybir
import concourse.tile as tile
from concourse.bass import ds, ts
from concourse.bass_utils import run_bass_kernel_spmd

F32 = mybir.dt.float32
BF16 = mybir.dt.bfloat16

B, S, D, H, DK = 4, 2048, 1024, 16, 64
THETA = 10000.0
NH = 8  # heads per core
HD = NH * DK  # 512 head dims per core
P = 128
NEG = -1.0e9
EXPF = mybir.ActivationFunctionType.Exp


def build_attention_nc(nrep=1):
    nc = bacc.Bacc("TRN2", target_bir_lowering=False, debug=False)

    xT = nc.dram_tensor("xT", [D, S], BF16, kind="ExternalInput")
    wqT = nc.dram_tensor("wqT", [D, HD], BF16, kind="ExternalInput")
    wkT = nc.dram_tensor("wkT", [D, HD], BF16, kind="ExternalInput")
    wvT = nc.dram_tensor("wvT", [D, HD], BF16, kind="ExternalInput")
    woT = nc.dram_tensor("woT", [HD, D], BF16, kind="ExternalInput")
    cosP = nc.dram_tensor("cosP", [P, S], BF16, kind="ExternalInput")
    sinP = nc.dram_tensor("sinP", [P, S], BF16, kind="ExternalInput")
    trimask = nc.dram_tensor("trimask", [P, P], BF16, kind="ExternalInput")
    negI = nc.dram_tensor("negI", [P, P], BF16, kind="ExternalInput")
    onesc = nc.dram_tensor("onesc", [P, P], BF16, kind="ExternalInput")
    outT = nc.dram_tensor("outT", [D, S], BF16, kind="ExternalOutput")
    rscr = nc.dram_tensor("rscr", [NH * 4, 512], BF16, kind="Internal")

    with tile.TileContext(nc) as tc:
        if nrep == 1:
            _attention_tile(
                tc, xT, wqT, wkT, wvT, woT, cosP, sinP, trimask, negI, onesc,
                outT, rscr,
            )
        else:
            with tc.For_i(0, nrep, 1):
                _attention_tile(
                    tc, xT, wqT, wkT, wvT, woT, cosP, sinP, trimask, negI,
                    onesc, outT, rscr,
                )
    nc.compile()
    return nc


def _attention_tile(tc, xT, wqT, wkT, wvT, woT, cosP, sinP, trimask, negI, onesc, outT, rscr):
    nc = tc.nc

    with tc.tile_pool(name="qkv", bufs=1) as qkv:
        # ---- persistent tiles ----
        x_sb = qkv.tile([P, 8, S], BF16, tag="x")     # [k%128, k//128, t]
        QT = qkv.tile([P, 4, S], BF16, tag="QT")      # [d'%128, d'//128, t]
        KT = qkv.tile([P, 4, S], BF16, tag="KT")
        VP = qkv.tile([P, 16, 528], BF16, tag="VP")   # [t%128, t//128, 8*(64+ones+pad)]
        HOP = qkv.tile([P, 4, S], BF16, tag="HOP")    # head pairs x [128 dv, t]
        cos_sb = qkv.tile([P, S], BF16, tag="cos")
        sin_sb = qkv.tile([P, S], BF16, tag="sin")
        wq_sb = qkv.tile([P, 8, HD], BF16, tag="wq")
        wk_sb = qkv.tile([P, 8, HD], BF16, tag="wk")
        wv_sb = qkv.tile([P, 8, HD], BF16, tag="wv")
        wo_sb = qkv.tile([P, 4, D], BF16, tag="wo")
        tri_sb = qkv.tile([P, P], BF16, tag="tri")
        negI_sb = qkv.tile([P, P], BF16, tag="negI")

        # ---- bulk input DMAs, all up front on the SP HWDGE queue ----
        nc.sync.dma_start(wq_sb, wqT.ap().rearrange("(o p) d -> p o d", p=P))
        nc.sync.dma_start(wk_sb, wkT.ap().rearrange("(o p) d -> p o d", p=P))
        xT_t = xT.ap().rearrange("(o p) t -> p o t", p=P)
        for tci in range(4):
            nc.sync.dma_start(x_sb[:, :, ds(tci * 512, 512)], xT_t[:, :, ds(tci * 512, 512)])
        nc.sync.dma_start(cos_sb, cosP.ap())
        nc.sync.dma_start(sin_sb, sinP.ap())
        nc.sync.dma_start(tri_sb, trimask.ap())
        nc.sync.dma_start(negI_sb, negI.ap())
        # ones columns of V' (col 64 of each 66-wide head group)
        vp_g = VP[:, :, :].rearrange("p k (h c) -> p k h c", c=66)
        nc.sync.dma_start(
            vp_g[:, :, :, 64:65],
            onesc.ap().rearrange("p (k h one) -> p k h one", k=16, one=1),
        )
        nc.sync.dma_start(wo_sb, woT.ap().rearrange("(m p) o -> p m o", p=P))
        nc.sync.dma_start(wv_sb, wvT.ap().rearrange("(o p) d -> p o d", p=P))

        # ---- phase A: Q then K (j-outer so RoPE starts early), then V ----
        with (
            tc.tile_pool(name="swpool", bufs=2) as swpool,
            tc.tile_pool(name="psA", bufs=6, space="PSUM") as psA,
        ):
            for dst, w_sb in ((QT, wq_sb), (KT, wk_sb)):
                for j in range(4):
                    for tci in range(4):
                        ps = psA.tile([P, 512], F32, tag="psA")
                        for i in range(8):
                            nc.tensor.matmul(
                                ps,
                                lhsT=w_sb[:, i, ts(j, P)],
                                rhs=x_sb[:, i, ds(tci * 512, 512)],
                                start=(i == 0),
                                stop=(i == 7),
                            )
                        nc.scalar.copy(dst[:, j, ds(tci * 512, 512)], ps)
                    # RoPE for this (tensor, j) column: pair-swap via 4
                    # partition-block DMAs (SP queue), then 3 DVE bf16 TTs
                    qsw = swpool.tile([P, S], BF16, tag="qsw")
                    for blk in range(4):
                        sb = blk + (1 if blk % 2 == 0 else -1)
                        nc.sync.dma_start(
                            qsw[blk * 32 : blk * 32 + 32, :],
                            dst[sb * 32 : sb * 32 + 32, j, :],
                        )
                    tmp = swpool.tile([P, S], BF16, tag="rtmp")
                    nc.vector.tensor_mul(tmp, cos_sb, dst[:, j, :])
                    nc.vector.tensor_mul(qsw, sin_sb, qsw)
                    nc.vector.tensor_add(dst[:, j, :], tmp, qsw)

            # V pass (x already resident)
            for tci in range(4):
                for tt in range(4):
                    ps = psA.tile([P, 512], F32, tag="psA")
                    for i in range(8):
                        nc.tensor.matmul(
                            ps,
                            lhsT=x_sb[:, i, ds(tci * 512 + tt * P, P)],
                            rhs=wv_sb[:, i, :],
                            start=(i == 0),
                            stop=(i == 7),
                        )
                    kt_idx = tci * 4 + tt
                    nc.scalar.copy(
                        vp_g[:, kt_idx, :, 0:64],
                        ps.rearrange("p (h c) -> p h c", c=64),
                    )

        # ---- phase B: attention per (qb, m) + interleaved phase C ----
        with (
            tc.tile_pool(name="ptile", bufs=3) as ptile,
            tc.tile_pool(name="srowp", bufs=4) as srowp,
            tc.tile_pool(name="scap", bufs=2) as scap,
            tc.tile_pool(name="evpool", bufs=2) as evpool,
            tc.tile_pool(name="obpool", bufs=3) as obpool,
            tc.tile_pool(name="psS", bufs=2, space="PSUM") as psS,
            tc.tile_pool(name="psPV", bufs=3, space="PSUM") as psPV,
            tc.tile_pool(name="psC", bufs=1, space="PSUM") as psC,
        ):
            outT_ap = outT.ap()
            pending_scale = []  # deferred HOP scale-mul: (m, qsl, sca)

            def flush_scale():
                while pending_scale:
                    pm, pqsl, psca = pending_scale.pop(0)
                    nc.vector.tensor_mul(HOP[:, pm, pqsl], HOP[:, pm, pqsl], psca)

            def c_group(cqb, ot, pool, tag):
                ps = pool.tile([P, 512], F32, tag=tag, name=f"c{cqb}_{ot}")
                for mm in range(4):
                    nc.tensor.matmul(
                        ps,
                        lhsT=wo_sb[:, mm, ts(ot, P)],
                        rhs=HOP[:, mm, ds(cqb * 512, 512)],
                        start=(mm == 0),
                        stop=(mm == 3),
                    )
                ob = obpool.tile([P, 512], BF16, tag="ob")
                nc.vector.tensor_copy(ob, ps)
                nc.sync.dma_start(outT_ap[ts(ot, P), ds(cqb * 512, 512)], ob)

            for qb in range(4):
                qsl = ds(qb * 512, 512)
                for m in range(4):
                    nkt = 4 * qb + 4
                    pvs = [
                        psPV.tile([P, 512], F32, tag="pv", name=f"pv{qb}{m}{e}")
                        for e in range(2)
                    ]

                    def pv_mms(pt2, c0, kt):
                        for e in range(2):
                            nc.tensor.matmul(
                                pvs[e][0:65, c0:],
                                lhsT=VP[:, kt, ds((2 * m + e) * 66, 65)],
                                rhs=pt2[:, e, c0:],
                                start=(kt == 0),
                                stop=(kt == nkt - 1),
                            )

                    prev = None
                    for kt in range(nkt):
                        roff = kt - 4 * qb
                        c0 = max(0, 128 * roff)
                        s2 = psS.tile([P, 2, 512], F32, tag="s")
                        for e in range(2):
                            rb = e * 64
                            if roff >= 0:
                                # diagonal 128-wide block: mask + scores
                                nc.tensor.matmul(
                                    s2[:, e, ds(c0, P)],
                                    lhsT=negI_sb,
                                    rhs=tri_sb,
                                    start=True,
                                    stop=False,
                                )
                                nc.tensor.matmul(
                                    s2[:, e, ds(c0, P)],
                                    lhsT=KT[rb : rb + 64, m, ts(kt, P)],
                                    rhs=QT[rb : rb + 64, m, ds(qb * 512 + c0, P)],
                                    start=False,
                                    stop=True,
                                )
                                if c0 + P < 512:
                                    nc.tensor.matmul(
                                        s2[:, e, ds(c0 + P, 384 - c0)],
                                        lhsT=KT[rb : rb + 64, m, ts(kt, P)],
                                        rhs=QT[rb : rb + 64, m, ds(qb * 512 + c0 + P, 384 - c0)],
                                        start=True,
                                        stop=True,
                                    )
                            else:
                                nc.tensor.matmul(
                                    s2[:, e, :],
                                    lhsT=KT[rb : rb + 64, m, ts(kt, P)],
                                    rhs=QT[rb : rb + 64, m, qsl],
                                    start=True,
                                    stop=True,
                                )
                        pt2 = ptile.tile([P, 2, 512], BF16, tag="pt")
                        nc.scalar.activation(
                            pt2[:, :, c0:], s2[:, :, c0:], EXPF, scale=0.125
                        )
                        if prev is not None:
                            pv_mms(*prev)
                        prev = (pt2, c0, kt)
                    pv_mms(*prev)

                    # deferred scale-mul of the previous m-block (its sca
                    # broadcast has long arrived), BEFORE this block's norm
                    # chain so phase-C matmuls never wait on fresh DVE work
                    flush_scale()

                    # normalizer: recip the sum row -> DRAM -> broadcast;
                    # evac pv unscaled
                    sca = scap.tile([P, 512], BF16, tag="sca")
                    for e in range(2):
                        pv = pvs[e]
                        srow = srowp.tile([1, 512], BF16, tag="srow")
                        with nc.allow_low_precision(reason="softmax normalizer to bf16"):
                            nc.vector.reciprocal(srow, pv[64:65, :])
                        slot = (2 * m + e) * 4 + qb
                        nc.gpsimd.dma_start(rscr.ap()[slot : slot + 1, :], srow)
                        rsrc = bass.AP(
                            tensor=rscr.ap().tensor,
                            offset=slot * 512,
                            ap=[[0, 64], [1, 512]],
                        )
                        nc.gpsimd.dma_start(sca[e * 64 : e * 64 + 64, :], rsrc)
                        if e == 0:
                            nc.vector.tensor_copy(HOP[0:64, m, qsl], pv[0:64, :])
                        else:
                            ev = evpool.tile([64, 512], BF16, tag="ev")
                            nc.vector.tensor_copy(ev, pv[0:64, :])
                            nc.gpsimd.dma_start(HOP[64:128, m, qsl], ev)
                    pending_scale.append((m, qsl, sca))

                    # interleave the previous qb's output projection here
                    if qb >= 1:
                        for ot in (2 * m, 2 * m + 1):
                            c_group(qb - 1, ot, psC, "psC")

            # tail: last qb's phase C, pipelined through spare psPV slots
            flush_scale()
            for ot in range(8):
                pool, tag = (psC, "psC") if ot % 4 == 0 else (psPV, "pv")
                c_group(3, ot, pool, tag)


# ---------------- host side ----------------

def _host_tables():
    import ml_dtypes

    i = np.arange(32, dtype=np.float32)
    inv_freq = (THETA ** (2.0 * i / DK)).astype(np.float32)
    t = np.arange(S, dtype=np.float32)
    ang = t[:, None] / inv_freq[None, :]  # [S, 32]
    c = np.cos(ang).astype(np.float32).T  # [32, S]
    sn = np.sin(ang).astype(np.float32).T
    cosP = np.tile(c, (4, 1))  # [128, S]
    sinP = np.tile(sn, (4, 1))
    sign = np.repeat(np.array([-1.0, 1.0, -1.0, 1.0], dtype=np.float32), 32)
    sinP = sinP * sign[:, None]

    kk = np.arange(P)[:, None]
    qq = np.arange(P)[None, :]
    trimask = (kk > qq).astype(ml_dtypes.bfloat16)  # [128,128]
    negI = (NEG * np.eye(P)).astype(ml_dtypes.bfloat16)
    bf = ml_dtypes.bfloat16
    return cosP.astype(bf), sinP.astype(bf), trimask, negI


_PERM = np.concatenate(
    [np.concatenate([h * 64 + np.arange(0, 64, 2), h * 64 + np.arange(1, 64, 2)])
     for h in range(NH)]
)

_NC_CACHE = {}


def make_in_maps(x, Wq, Wk, Wv, Wo):
    import ml_dtypes

    bf = ml_dtypes.bfloat16
    cosP, sinP, trimask, negI = _host_tables()
    in_maps = []
    for c in range(8):
        b, hh = c // 2, c % 2
        sl = slice(hh * HD, (hh + 1) * HD)
        in_maps.append(
            {
                "xT": np.ascontiguousarray(x[b].T).astype(bf),
                "wqT": np.ascontiguousarray(Wq[sl, :][_PERM].T).astype(bf),
                "wkT": np.ascontiguousarray(Wk[sl, :][_PERM].T).astype(bf),
                "wvT": np.ascontiguousarray(Wv[sl, :].T).astype(bf),
                "woT": np.ascontiguousarray(Wo[:, sl].T).astype(bf),
                "cosP": cosP,
                "sinP": sinP,
                "trimask": trimask,
                "negI": negI,
                "onesc": np.ones((P, P), dtype=bf),
            }
        )
    return in_maps


def gather_out(core_outs):
    out = np.empty((B, S, D), dtype=np.float32)
    for b in range(B):
        a = np.asarray(core_outs[2 * b]["outT"], dtype=np.float32)
        bb = np.asarray(core_outs[2 * b + 1]["outT"], dtype=np.float32)
        out[b] = (a + bb).T
    return out


def kernel(x, Wq, Wk, Wv, Wo):
    x = np.asarray(x, dtype=np.float32)
    Wq = np.asarray(Wq, dtype=np.float32)
    Wk = np.asarray(Wk, dtype=np.float32)
    Wv = np.asarray(Wv, dtype=np.float32)
    Wo = np.asarray(Wo, dtype=np.float32)

    if "nc" not in _NC_CACHE:
        _NC_CACHE["nc"] = build_attention_nc()
    nc = _NC_CACHE["nc"]

    in_maps = make_in_maps(x, Wq, Wk, Wv, Wo)
    res = run_bass_kernel_spmd(nc, in_maps, core_ids=list(range(8)))
    return gather_out(res.results)


# revision 23
# speedup vs baseline: 1.5152x; 1.1518x over previous
"""Causal multi-head attention (B=4, S=2048, D=1024, H=16, RoPE) on 8 trn2 cores.

Sharding: core c -> batch c//2, head-half c%2 (8 heads / 512 dims per core).
Each core computes QKV projections for its head slice, RoPE, causal flash
attention, and a partial output projection with its Wo column slice; the host
sums the two partials per batch (the tensor-parallel all-reduce) and
transposes back.

v2 layout/scheduling (vs the 406us baseline):
  - bf16 everywhere outside PSUM accumulation (inputs host-cast): same PE
    rate as fp32r but no 256-wide fp32r floor, half the DMA bytes, and 2x
    DVE throughput on the elementwise ops
  - x loaded ONCE into SBUF (32KB/partition) and reused for the V pass
  - pass 1 is j-outer so each (tensor, j) head-pair column finishes early;
    RoPE (swap-DMA + 3 DVE TTs) runs per (tensor, j) overlapped with the
    rest of pass 1 / the V pass; PSUM evacuations all on ACT (idle there)
  - causal mask matmuls cover only the 128-wide diagonal block (bf16 S
    matmuls don't need the fp32r >=256 moving width)
  - PV matmuls are issued one kt behind the S/exp pair so PE never waits
    on the exp latency; ACT carries exps only
  - softmax normalizer: ones-column fused in V' gives the row sum; DVE
    reciprocal -> DRAM round-trip broadcast on the Pool SWDGE queue (keeps
    the shared HWDGE free); the scale-multiply is deferred one m-block
  - phase C (output projection) groups are interleaved into the NEXT
    q-block's attention so the PSUM evac never stalls PE; the last qb's
    groups drain through spare psPV slots
"""

import numpy as np

import concourse.bass as bass
import concourse.bacc as bacc
import concourse.mybir as mybir
import concourse.tile as tile
from concourse.bass import ds, ts
from concourse.bass_utils import run_bass_kernel_spmd

F32 = mybir.dt.float32
BF16 = mybir.dt.bfloat16

B, S, D, H, DK = 4, 2048, 1024, 16, 64
THETA = 10000.0
NH = 8  # heads per core
HD = NH * DK  # 512 head dims per core
P = 128
NEG = -1.0e9
EXPF = mybir.ActivationFunctionType.Exp


def build_attention_nc(nrep=1):
    nc = bacc.Bacc("TRN2", target_bir_lowering=False, debug=False)

    xT = nc.dram_tensor("xT", [D, S], BF16, kind="ExternalInput")
    wqT = nc.dram_tensor("wqT", [D, HD], BF16, kind="ExternalInput")
    wkT = nc.dram_tensor("wkT", [D, HD], BF16, kind="ExternalInput")
    wvT = nc.dram_tensor("wvT", [D, HD], BF16, kind="ExternalInput")
    woT = nc.dram_tensor("woT", [HD, D], BF16, kind="ExternalInput")
    cosP = nc.dram_tensor("cosP", [P, S], BF16, kind="ExternalInput")
    sinP = nc.dram_tensor("sinP", [P, S], BF16, kind="ExternalInput")
    trimask = nc.dram_tensor("trimask", [P, 2 * P], BF16, kind="ExternalInput")
    onesc = nc.dram_tensor("onesc", [P, P], BF16, kind="ExternalInput")
    outT = nc.dram_tensor("outT", [D, S], BF16, kind="ExternalOutput")
    rscr = nc.dram_tensor("rscr", [NH * 4, 512], BF16, kind="Internal")

    with tile.TileContext(nc) as tc:
        if nrep == 1:
            _attention_tile(
                tc, xT, wqT, wkT, wvT, woT, cosP, sinP, trimask, onesc,
                outT, rscr,
            )
        else:
            with tc.For_i(0, nrep, 1):
                _attention_tile(
                    tc, xT, wqT, wkT, wvT, woT, cosP, sinP, trimask,
                    onesc, outT, rscr,
                )
    nc.compile()
    return nc


def _attention_tile(tc, xT, wqT, wkT, wvT, woT, cosP, sinP, trimask, onesc, outT, rscr):
    nc = tc.nc

    with tc.tile_pool(name="qkv", bufs=1) as qkv:
        # ---- persistent tiles ----
        x_sb = qkv.tile([P, 8, S], BF16, tag="x")     # [k%128, k//128, t]
        QT = qkv.tile([P, 4, S], BF16, tag="QT")      # [d'%128, d'//128, t]
        KT = qkv.tile([P, 4, S], BF16, tag="KT")
        VP = qkv.tile([P, 16, 528], BF16, tag="VP")   # [t%128, t//128, 8*(64+ones+pad)]
        HOP = qkv.tile([P, 4, S], BF16, tag="HOP")    # head pairs x [128 dv, t]
        cos_sb = qkv.tile([P, S], BF16, tag="cos")
        sin_sb = qkv.tile([P, S], BF16, tag="sin")
        wq_sb = qkv.tile([P, 8, HD], BF16, tag="wq")
        wk_sb = qkv.tile([P, 8, HD], BF16, tag="wk")
        wv_sb = qkv.tile([P, 8, HD], BF16, tag="wv")
        wo_sb = qkv.tile([P, 4, D], BF16, tag="wo")
        tri_sb = qkv.tile([P, 2, P], BF16, tag="tri")

        ones_sb = qkv.tile([1, P], BF16, tag="ones1")

        # ---- bulk input DMAs, all up front on the SP HWDGE queue, in
        # first-use order (x chunk 0 and wq gate the first matmul) ----
        xT_t = xT.ap().rearrange("(o p) t -> p o t", p=P)
        wq_src = wqT.ap().rearrange("(o p) d -> p o d", p=P)
        # first mm group needs x chunk 0 + wq; split both so the halves land
        # (and the first 4-step accumulation starts) as early as possible
        nc.sync.dma_start(wq_sb[:, 0:2, :], wq_src[:, 0:2, :])
        nc.sync.dma_start(x_sb[:, 0:2, ds(0, 512)], xT_t[:, 0:2, ds(0, 512)])
        nc.sync.dma_start(wq_sb[:, 2:4, :], wq_src[:, 2:4, :])
        nc.sync.dma_start(x_sb[:, 2:4, ds(0, 512)], xT_t[:, 2:4, ds(0, 512)])
        nc.sync.dma_start(wq_sb[:, 4:8, :], wq_src[:, 4:8, :])
        nc.sync.dma_start(x_sb[:, 4:8, ds(0, 512)], xT_t[:, 4:8, ds(0, 512)])
        nc.sync.dma_start(wk_sb, wkT.ap().rearrange("(o p) d -> p o d", p=P))
        for tci in range(1, 4):
            nc.sync.dma_start(x_sb[:, :, ds(tci * 512, 512)], xT_t[:, :, ds(tci * 512, 512)])
        nc.sync.dma_start(cos_sb, cosP.ap())
        nc.sync.dma_start(sin_sb, sinP.ap())
        nc.sync.dma_start(tri_sb, trimask.ap().rearrange("p (e q) -> p e q", e=2))
        nc.sync.dma_start(ones_sb, onesc.ap()[0:1, :])
        # ones columns of V' (col 64 of each 66-wide head group)
        vp_g = VP[:, :, :].rearrange("p k (h c) -> p k h c", c=66)
        nc.sync.dma_start(
            vp_g[:, :, :, 64:65],
            onesc.ap().rearrange("p (k h one) -> p k h one", k=16, one=1),
        )
        nc.sync.dma_start(wo_sb, woT.ap().rearrange("(m p) o -> p m o", p=P))
        nc.sync.dma_start(wv_sb, wvT.ap().rearrange("(o p) d -> p o d", p=P))

        # ---- phase A: Q+K (tci-outer matches x arrival; RoPE per (dst,j)
        # once its last chunk lands), then V ----
        with (
            tc.tile_pool(name="swpool", bufs=2) as swpool,
            tc.tile_pool(name="psA", bufs=6, space="PSUM") as psA,
        ):
            for tci in range(4):
                for dst, w_sb in ((QT, wq_sb), (KT, wk_sb)):
                    for j in range(4):
                        ps = psA.tile([P, 512], F32, tag="psA")
                        for i in range(8):
                            nc.tensor.matmul(
                                ps,
                                lhsT=w_sb[:, i, ts(j, P)],
                                rhs=x_sb[:, i, ds(tci * 512, 512)],
                                start=(i == 0),
                                stop=(i == 7),
                            )
                        nc.scalar.copy(dst[:, j, ds(tci * 512, 512)], ps)
                        if tci == 3:
                            # RoPE for this (tensor, j) column: pair-swap via
                            # 4 partition-block DMAs (SP queue), 3 DVE bf16 TTs
                            qsw = swpool.tile([P, S], BF16, tag="qsw")
                            for blk in range(4):
                                sb = blk + (1 if blk % 2 == 0 else -1)
                                nc.sync.dma_start(
                                    qsw[blk * 32 : blk * 32 + 32, :],
                                    dst[sb * 32 : sb * 32 + 32, j, :],
                                )
                            tmp = swpool.tile([P, S], BF16, tag="rtmp")
                            nc.vector.tensor_mul(tmp, cos_sb, dst[:, j, :])
                            nc.vector.tensor_mul(qsw, sin_sb, qsw)
                            nc.vector.tensor_add(dst[:, j, :], tmp, qsw)

        # ---- phase B: attention per (qb, m) + interleaved phase C ----
        with (
            tc.tile_pool(name="ptile", bufs=4) as ptile,
            tc.tile_pool(name="srowp", bufs=4) as srowp,
            tc.tile_pool(name="scap", bufs=2) as scap,
            tc.tile_pool(name="obpool", bufs=3) as obpool,
            tc.tile_pool(name="psS", bufs=2, space="PSUM") as psS,
            tc.tile_pool(name="psPV", bufs=4, space="PSUM") as psPV,
        ):
            psC = psPV  # phase-C groups share the 4-slot ring
            outT_ap = outT.ap()
            pending_scale = []  # deferred HOP scale-mul: (m, qsl, sca)

            def flush_scale():
                while pending_scale:
                    pm, pqsl, psca = pending_scale.pop(0)
                    nc.vector.tensor_mul(HOP[:, pm, pqsl], HOP[:, pm, pqsl], psca)

            def c_group(cqb, ot, tail=False):
                ps = psC.tile([P, 512], F32, tag="pv", name=f"c{cqb}_{ot}")
                for mm in range(4):
                    nc.tensor.matmul(
                        ps,
                        lhsT=wo_sb[:, mm, ts(ot, P)],
                        rhs=HOP[:, mm, ds(cqb * 512, 512)],
                        start=(mm == 0),
                        stop=(mm == 3),
                    )
                ob = obpool.tile([P, 512], BF16, tag="ob")
                # at the tail spread evacs/stores over two engines/queues so
                # the final drain pipelines; outT stores ride the Pool SWDGE
                # to stay off the shared HWDGE
                if tail and ot % 2:
                    nc.scalar.copy(ob, ps)
                else:
                    nc.vector.tensor_copy(ob, ps)
                dma_eng = nc.sync if tail and ot % 2 else nc.gpsimd
                dma_eng.dma_start(outT_ap[ts(ot, P), ds(cqb * 512, 512)], ob)

            def s_exp_unit(qb, m, kt):
                """S matmuls + exp for one (qb, m, kt) unit; returns PV args."""
                roff = kt - 4 * qb
                c0 = max(0, 128 * roff)
                qsl = ds(qb * 512, 512)
                s2 = psS.tile([P, 2, 512], F32, tag="s")
                for e in range(2):
                    rb = e * 64
                    nc.tensor.matmul(
                        s2[:, e, c0:],
                        lhsT=KT[rb : rb + 64, m, ts(kt, P)],
                        rhs=QT[rb : rb + 64, m, ds(qb * 512 + c0, 512 - c0)],
                        start=True,
                        stop=True,
                    )
                pt2 = ptile.tile([P, 2, 512], BF16, tag="pt")
                nc.scalar.activation(pt2[:, :, c0:], s2[:, :, c0:], EXPF, scale=0.125)
                if roff >= 0:
                    # causal mask: zero the upper triangle of the diagonal
                    # 128-block multiplicatively (post-exp, bf16 on DVE)
                    nc.vector.tensor_mul(
                        pt2[:, :, ds(c0, P)], pt2[:, :, ds(c0, P)], tri_sb
                    )
                return pt2, c0

            blk_pvs = {}  # (qb, m) -> pv tile pair

            def pv_unit(qb, m, kt, pt2, c0):
                nkt = 4 * qb + 4
                if kt == 0:
                    blk_pvs[(qb, m)] = [
                        psPV.tile([P, 512], F32, tag="pv", name=f"pv{qb}{m}{e}")
                        for e in range(2)
                    ]
                pvs = blk_pvs[(qb, m)]
                for e in range(2):
                    nc.tensor.matmul(
                        pvs[e][0:65, c0:],
                        lhsT=VP[:, kt, ds((2 * m + e) * 66, 65)],
                        rhs=pt2[:, e, c0:],
                        start=(kt == 0),
                        stop=(kt == nkt - 1),
                    )

            def norm_block(qb, m, tail=False):
                """Normalizer + pv evac for a finished (qb, m) block, then the
                scheduled phase-C groups of qb-1. Both recip rows go out in
                one DVE-queue DMA; one SP-queue DMA broadcasts them back from
                DRAM to 2x64 partitions. pv evac is two direct DVE copies
                (DVE handles the 0->64 partition shift for e1)."""
                qsl = ds(qb * 512, 512)
                pvs = blk_pvs.pop((qb, m))
                flush_scale()
                if qb >= 1:
                    for ot in (2 * m, 2 * m + 1):
                        c_group(qb - 1, ot)
                sca = scap.tile([P, 512], BF16, tag="sca")
                if not tail:
                    slot = 8 * qb + 2 * m
                    for e in range(2):
                        srow = srowp.tile([1, 512], BF16, tag="srow1")
                        with nc.allow_low_precision(reason="softmax normalizer to bf16"):
                            nc.vector.reciprocal(srow, pvs[e][64:65, :])
                        nc.gpsimd.dma_start(rscr.ap()[slot + e : slot + e + 1, :], srow)
                    rsrc = bass.AP(
                        tensor=rscr.ap().tensor,
                        offset=slot * 512,
                        ap=[[512, 2], [0, 64], [1, 512]],
                    )
                    nc.sync.dma_start(sca, rsrc)
                nc.vector.tensor_copy(HOP[0:64, m, qsl], pvs[0][0:64, :])
                nc.vector.tensor_copy(HOP[64:128, m, qsl], pvs[1][0:64, :])
                if tail:
                    # latency-critical last block: broadcast the recip rows
                    # with PE matmuls instead of the DRAM round trip
                    bc = psS.tile([P, 2, 512], F32, tag="s", name="bcast")
                    for e in range(2):
                        srow = srowp.tile([1, 512], BF16, tag="srow1")
                        with nc.allow_low_precision(reason="softmax normalizer to bf16"):
                            nc.vector.reciprocal(srow, pvs[e][64:65, :])
                        nc.tensor.matmul(
                            bc[e * 64 : e * 64 + 64, 0, :],
                            lhsT=ones_sb[0:1, 0:64],
                            rhs=srow,
                            start=True,
                            stop=True,
                        )
                    nc.vector.tensor_copy(sca, bc[:, 0, :])
                    nc.vector.tensor_mul(HOP[:, m, qsl], HOP[:, m, qsl], sca)
                else:
                    pending_scale.append((m, qsl, sca))

            # flat software pipeline over all (qb, m, kt) units: PV matmuls
            # lag the S/exp pair by two units so PE never waits on exp latency
            units = [
                (qb, m, kt)
                for qb in range(4)
                for m in range(4)
                for kt in range(4 * qb + 4)
            ]

            pipe = []

            def drain_one():
                q_, m_, k_, p_, c_ = pipe.pop(0)
                pv_unit(q_, m_, k_, p_, c_)
                if k_ == 4 * q_ + 3:
                    norm_block(q_, m_, tail=(q_ == 3 and m_ == 3))

            for qb, m, kt in units:
                pt2, c0 = s_exp_unit(qb, m, kt)
                pipe.append((qb, m, kt, pt2, c0))
                if len(pipe) > 2:
                    drain_one()
            while pipe:
                drain_one()

            # tail: last qb's phase C through the shared PSUM ring
            for ot in range(8):
                c_group(3, ot, tail=True)


# ---------------- host side ----------------

def _host_tables():
    import ml_dtypes

    i = np.arange(32, dtype=np.float32)
    inv_freq = (THETA ** (2.0 * i / DK)).astype(np.float32)
    t = np.arange(S, dtype=np.float32)
    ang = t[:, None] / inv_freq[None, :]  # [S, 32]
    c = np.cos(ang).astype(np.float32).T  # [32, S]
    sn = np.sin(ang).astype(np.float32).T
    cosP = np.tile(c, (4, 1))  # [128, S]
    sinP = np.tile(sn, (4, 1))
    sign = np.repeat(np.array([-1.0, 1.0, -1.0, 1.0], dtype=np.float32), 32)
    sinP = sinP * sign[:, None]

    kk = np.arange(P)[:, None]
    qq = np.arange(P)[None, :]
    keep = (kk <= qq).astype(ml_dtypes.bfloat16)  # [128,128]
    trimask = np.tile(keep, (1, 2))  # [128, 2*128] (both head halves)
    bf = ml_dtypes.bfloat16
    return cosP.astype(bf), sinP.astype(bf), trimask


_PERM = np.concatenate(
    [np.concatenate([h * 64 + np.arange(0, 64, 2), h * 64 + np.arange(1, 64, 2)])
     for h in range(NH)]
)

_NC_CACHE = {}


def make_in_maps(x, Wq, Wk, Wv, Wo):
    import ml_dtypes

    bf = ml_dtypes.bfloat16
    cosP, sinP, trimask = _host_tables()
    in_maps = []
    for c in range(8):
        b, hh = c // 2, c % 2
        sl = slice(hh * HD, (hh + 1) * HD)
        in_maps.append(
            {
                "xT": np.ascontiguousarray(x[b].T).astype(bf),
                "wqT": np.ascontiguousarray(Wq[sl, :][_PERM].T).astype(bf),
                "wkT": np.ascontiguousarray(Wk[sl, :][_PERM].T).astype(bf),
                "wvT": np.ascontiguousarray(Wv[sl, :].T).astype(bf),
                "woT": np.ascontiguousarray(Wo[:, sl].T).astype(bf),
                "cosP": cosP,
                "sinP": sinP,
                "trimask": trimask,
                "onesc": np.ones((P, P), dtype=bf),
            }
        )
    return in_maps


def gather_out(core_outs):
    out = np.empty((B, S, D), dtype=np.float32)
    for b in range(B):
        a = np.asarray(core_outs[2 * b]["outT"], dtype=np.float32)
        bb = np.asarray(core_outs[2 * b + 1]["outT"], dtype=np.float32)
        out[b] = (a + bb).T
    return out


def kernel(x, Wq, Wk, Wv, Wo):
    x = np.asarray(x, dtype=np.float32)
    Wq = np.asarray(Wq, dtype=np.float32)
    Wk = np.asarray(Wk, dtype=np.float32)
    Wv = np.asarray(Wv, dtype=np.float32)
    Wo = np.asarray(Wo, dtype=np.float32)

    if "nc" not in _NC_CACHE:
        _NC_CACHE["nc"] = build_attention_nc()
    nc = _NC_CACHE["nc"]

    in_maps = make_in_maps(x, Wq, Wk, Wv, Wo)
    res = run_bass_kernel_spmd(nc, in_maps, core_ids=list(range(8)))
    return gather_out(res.results)


# revision 42
# speedup vs baseline: 1.5482x; 1.0218x over previous
"""Causal multi-head attention (B=4, S=2048, D=1024, H=16, RoPE) on 8 trn2 cores.

Sharding: core c -> batch c//2, head-half c%2 (8 heads / 512 dims per core).
Each core computes QKV projections for its head slice, RoPE, causal flash
attention, and a partial output projection with its Wo column slice; the host
sums the two partials per batch (the tensor-parallel all-reduce) and
transposes back.

v2 layout/scheduling (vs the 406us baseline):
  - bf16 everywhere outside PSUM accumulation (inputs host-cast): same PE
    rate as fp32r but no 256-wide fp32r floor, half the DMA bytes, and 2x
    DVE throughput on the elementwise ops
  - x loaded ONCE into SBUF (32KB/partition) and reused for the V pass
  - pass 1 is j-outer so each (tensor, j) head-pair column finishes early;
    RoPE (swap-DMA + 3 DVE TTs) runs per (tensor, j) overlapped with the
    rest of pass 1 / the V pass; PSUM evacuations all on ACT (idle there)
  - causal mask matmuls cover only the 128-wide diagonal block (bf16 S
    matmuls don't need the fp32r >=256 moving width)
  - PV matmuls are issued one kt behind the S/exp pair so PE never waits
    on the exp latency; ACT carries exps only
  - softmax normalizer: ones-column fused in V' gives the row sum; DVE
    reciprocal -> DRAM round-trip broadcast on the Pool SWDGE queue (keeps
    the shared HWDGE free); the scale-multiply is deferred one m-block
  - phase C (output projection) groups are interleaved into the NEXT
    q-block's attention so the PSUM evac never stalls PE; the last qb's
    groups drain through spare psPV slots
"""

import numpy as np

import concourse.bass as bass
import concourse.bacc as bacc
import concourse.mybir as mybir
import concourse.tile as tile
from concourse.bass import ds, ts
from concourse.bass_utils import run_bass_kernel_spmd

F32 = mybir.dt.float32
BF16 = mybir.dt.bfloat16

B, S, D, H, DK = 4, 2048, 1024, 16, 64
THETA = 10000.0
NH = 8  # heads per core
HD = NH * DK  # 512 head dims per core
P = 128
NEG = -1.0e9
EXPF = mybir.ActivationFunctionType.Exp


def build_attention_nc(nrep=1):
    nc = bacc.Bacc("TRN2", target_bir_lowering=False, debug=False)

    xT = nc.dram_tensor("xT", [D, S], BF16, kind="ExternalInput")
    wqT = nc.dram_tensor("wqT", [D, HD], BF16, kind="ExternalInput")
    wkT = nc.dram_tensor("wkT", [D, HD], BF16, kind="ExternalInput")
    wvT = nc.dram_tensor("wvT", [D, HD], BF16, kind="ExternalInput")
    woT = nc.dram_tensor("woT", [HD, D], BF16, kind="ExternalInput")
    cosP = nc.dram_tensor("cosP", [P, S], BF16, kind="ExternalInput")
    sinP = nc.dram_tensor("sinP", [P, S], BF16, kind="ExternalInput")
    trimask = nc.dram_tensor("trimask", [P, 2 * P], BF16, kind="ExternalInput")
    onesc = nc.dram_tensor("onesc", [P, P], BF16, kind="ExternalInput")
    outT = nc.dram_tensor("outT", [D, S], BF16, kind="ExternalOutput")
    rscr = nc.dram_tensor("rscr", [NH * 4, 512], BF16, kind="Internal")

    with tile.TileContext(nc) as tc:
        if nrep == 1:
            _attention_tile(
                tc, xT, wqT, wkT, wvT, woT, cosP, sinP, trimask, onesc,
                outT, rscr,
            )
        else:
            with tc.For_i(0, nrep, 1):
                _attention_tile(
                    tc, xT, wqT, wkT, wvT, woT, cosP, sinP, trimask,
                    onesc, outT, rscr,
                )
    nc.compile()
    return nc


def _attention_tile(tc, xT, wqT, wkT, wvT, woT, cosP, sinP, trimask, onesc, outT, rscr):
    nc = tc.nc

    with tc.tile_pool(name="qkv", bufs=1) as qkv:
        # ---- persistent tiles ----
        x_sb = qkv.tile([P, 8, S], BF16, tag="x")     # [k%128, k//128, t]
        QT = qkv.tile([P, 4, S], BF16, tag="QT")      # [d'%128, d'//128, t]
        KT = qkv.tile([P, 4, S], BF16, tag="KT")
        VP = qkv.tile([P, 16, 528], BF16, tag="VP")   # [t%128, t//128, 8*(64+ones+pad)]
        HOP = qkv.tile([P, 4, S], BF16, tag="HOP")    # head pairs x [128 dv, t]
        cos_sb = qkv.tile([P, S], BF16, tag="cos")
        sin_sb = qkv.tile([P, S], BF16, tag="sin")
        wq_sb = qkv.tile([P, 8, HD], BF16, tag="wq")
        wk_sb = qkv.tile([P, 8, HD], BF16, tag="wk")
        wv_sb = qkv.tile([P, 8, HD], BF16, tag="wv")
        wo_sb = qkv.tile([P, 4, D], BF16, tag="wo")
        tri_sb = qkv.tile([P, 2, P], BF16, tag="tri")

        ones_sb = qkv.tile([1, P], BF16, tag="ones1")

        # ---- bulk input DMAs, all up front on the SP HWDGE queue, in
        # first-use order (x chunk 0 and wq gate the first matmul) ----
        xT_t = xT.ap().rearrange("(o p) t -> p o t", p=P)
        wq_src = wqT.ap().rearrange("(o p) d -> p o d", p=P)
        # first mm group needs x chunk 0 + wq; split both so the halves land
        # (and the first 4-step accumulation starts) as early as possible
        # wq pieces ride the ACT HWDGE queue, x pieces the SP queue, so the
        # two issue pipelines overlap and the first 2-block accumulation can
        # start as soon as its operands land
        nc.sync.dma_start(wq_sb[:, 0:2, :], wq_src[:, 0:2, :])
        nc.scalar.dma_start(x_sb[:, 0:2, ds(0, 512)], xT_t[:, 0:2, ds(0, 512)])
        nc.sync.dma_start(wq_sb[:, 2:4, :], wq_src[:, 2:4, :])
        nc.scalar.dma_start(x_sb[:, 2:4, ds(0, 512)], xT_t[:, 2:4, ds(0, 512)])
        nc.sync.dma_start(wq_sb[:, 4:8, :], wq_src[:, 4:8, :])
        nc.scalar.dma_start(x_sb[:, 4:8, ds(0, 512)], xT_t[:, 4:8, ds(0, 512)])
        nc.scalar.dma_start(wk_sb, wkT.ap().rearrange("(o p) d -> p o d", p=P))
        for tci in range(1, 4):
            nc.sync.dma_start(x_sb[:, :, ds(tci * 512, 512)], xT_t[:, :, ds(tci * 512, 512)])
        nc.sync.dma_start(cos_sb, cosP.ap())
        nc.sync.dma_start(sin_sb, sinP.ap())
        nc.sync.dma_start(tri_sb, trimask.ap().rearrange("p (e q) -> p e q", e=2))
        nc.sync.dma_start(ones_sb, onesc.ap()[0:1, :])
        # ones columns of V' (col 64 of each 66-wide head group)
        vp_g = VP[:, :, :].rearrange("p k (h c) -> p k h c", c=66)
        nc.sync.dma_start(
            vp_g[:, :, :, 64:65],
            onesc.ap().rearrange("p (k h one) -> p k h one", k=16, one=1),
        )
        nc.sync.dma_start(wo_sb, woT.ap().rearrange("(m p) o -> p m o", p=P))
        nc.sync.dma_start(wv_sb, wvT.ap().rearrange("(o p) d -> p o d", p=P))

        # ---- phase A: Q+K (tci-outer matches x arrival; RoPE per (dst,j)
        # once its last chunk lands), then V ----
        with (
            tc.tile_pool(name="swpool", bufs=2) as swpool,
            tc.tile_pool(name="psA", bufs=6, space="PSUM") as psA,
        ):
            for tci in range(4):
                for dst, w_sb in ((QT, wq_sb), (KT, wk_sb)):
                    for j in range(4):
                        ps = psA.tile([P, 512], F32, tag="psA")
                        for i in range(8):
                            nc.tensor.matmul(
                                ps,
                                lhsT=w_sb[:, i, ts(j, P)],
                                rhs=x_sb[:, i, ds(tci * 512, 512)],
                                start=(i == 0),
                                stop=(i == 7),
                            )
                        nc.scalar.copy(dst[:, j, ds(tci * 512, 512)], ps)
                        if tci % 2 == 1:
                            # RoPE for this (tensor, j) half-row as soon as
                            # both its chunks exist: pair-swap via 4
                            # partition-block DMAs (Q on the SP queue, K on
                            # the ACT queue so neither backs up), 3 DVE bf16
                            # TTs. Half-row granularity spreads the DVE work
                            # across pass 1 instead of piling it at the end.
                            hsl = ds((tci // 2) * 1024, 1024)
                            qsw = swpool.tile([P, 1024], BF16, tag="qsw")
                            for blk in range(4):
                                sb = blk + (1 if blk % 2 == 0 else -1)
                                nc.sync.dma_start(
                                    qsw[blk * 32 : blk * 32 + 32, :],
                                    dst[sb * 32 : sb * 32 + 32, j, hsl],
                                )
                            tmp = swpool.tile([P, 1024], BF16, tag="rtmp")
                            nc.vector.tensor_mul(tmp, cos_sb[:, hsl], dst[:, j, hsl])
                            nc.vector.tensor_mul(qsw, sin_sb[:, hsl], qsw)
                            nc.vector.tensor_add(dst[:, j, hsl], tmp, qsw)

            # V pass (x already resident)
            for tci in range(4):
                for tt in range(4):
                    ps = psA.tile([P, 512], F32, tag="psA")
                    for i in range(8):
                        nc.tensor.matmul(
                            ps,
                            lhsT=x_sb[:, i, ds(tci * 512 + tt * P, P)],
                            rhs=wv_sb[:, i, :],
                            start=(i == 0),
                            stop=(i == 7),
                        )
                    kt_idx = tci * 4 + tt
                    nc.scalar.copy(
                        vp_g[:, kt_idx, :, 0:64],
                        ps.rearrange("p (h c) -> p h c", c=64),
                    )

        # ---- phase B: attention per (qb, m) + interleaved phase C ----
        with (
            tc.tile_pool(name="ptile", bufs=4) as ptile,
            tc.tile_pool(name="srowp", bufs=4) as srowp,
            tc.tile_pool(name="scap", bufs=2) as scap,
            tc.tile_pool(name="obpool", bufs=3) as obpool,
            tc.tile_pool(name="psS", bufs=2, space="PSUM") as psS,
            tc.tile_pool(name="psPV", bufs=4, space="PSUM") as psPV,
        ):
            psC = psPV  # phase-C groups share the 4-slot ring
            outT_ap = outT.ap()
            pending_scale = []  # deferred HOP scale-mul: (m, qsl, sca)

            def flush_scale():
                while pending_scale:
                    pm, pqsl, psca = pending_scale.pop(0)
                    nc.vector.tensor_mul(HOP[:, pm, pqsl], HOP[:, pm, pqsl], psca)

            outT_t = outT_ap.rearrange("(o p) t -> p o t", p=P)

            def c_pair(cqb, ot0, tail=False):
                """Two output-projection column groups sharing one ob tile
                and one paired outT store (halves the store DMA count)."""
                ob2 = obpool.tile([P, 2, 512], BF16, tag="ob")
                for k in range(2):
                    ot = ot0 + k
                    ps = psC.tile([P, 512], F32, tag="pv", name=f"c{cqb}_{ot}")
                    for mm in range(4):
                        nc.tensor.matmul(
                            ps,
                            lhsT=wo_sb[:, mm, ts(ot, P)],
                            rhs=HOP[:, mm, ds(cqb * 512, 512)],
                            start=(mm == 0),
                            stop=(mm == 3),
                        )
                    # at the tail split evacs over ACT+DVE so the final
                    # drain pipelines
                    if tail and k:
                        nc.scalar.copy(ob2[:, k, :], ps)
                    else:
                        nc.vector.tensor_copy(ob2[:, k, :], ps)
                dma_eng = nc.sync if tail and (ot0 // 2) % 2 else nc.gpsimd
                dma_eng.dma_start(
                    outT_t[:, ot0 : ot0 + 2, ds(cqb * 512, 512)], ob2
                )

            def s_exp_unit(qb, m, kt):
                """S matmuls + exp for one (qb, m, kt) unit; returns PV args."""
                roff = kt - 4 * qb
                c0 = max(0, 128 * roff)
                qsl = ds(qb * 512, 512)
                s2 = psS.tile([P, 2, 512], F32, tag="s")
                for e in range(2):
                    rb = e * 64
                    nc.tensor.matmul(
                        s2[:, e, c0:],
                        lhsT=KT[rb : rb + 64, m, ts(kt, P)],
                        rhs=QT[rb : rb + 64, m, ds(qb * 512 + c0, 512 - c0)],
                        start=True,
                        stop=True,
                    )
                pt2 = ptile.tile([P, 2, 512], BF16, tag="pt")
                nc.scalar.activation(pt2[:, :, c0:], s2[:, :, c0:], EXPF, scale=0.125)
                if roff >= 0:
                    # causal mask: zero the upper triangle of the diagonal
                    # 128-block multiplicatively post-exp. GPSIMD, not DVE:
                    # the DVE queue carries the norm chains, whose head-of-
                    # line waits would delay the mask and stall the PV
                    nc.vector.tensor_mul(
                        pt2[:, :, ds(c0, P)], pt2[:, :, ds(c0, P)], tri_sb
                    )
                return pt2, c0

            blk_pvs = {}  # (qb, m) -> pv tile pair

            def pv_unit(qb, m, kt, pt2, c0):
                nkt = 4 * qb + 4
                if kt == 0:
                    blk_pvs[(qb, m)] = [
                        psPV.tile([P, 512], F32, tag="pv", name=f"pv{qb}{m}{e}")
                        for e in range(2)
                    ]
                pvs = blk_pvs[(qb, m)]
                for e in range(2):
                    nc.tensor.matmul(
                        pvs[e][0:65, c0:],
                        lhsT=VP[:, kt, ds((2 * m + e) * 66, 65)],
                        rhs=pt2[:, e, c0:],
                        start=(kt == 0),
                        stop=(kt == nkt - 1),
                    )

            def norm_block(qb, m, tail=False):
                """Normalizer + pv evac for a finished (qb, m) block, then the
                scheduled phase-C pair of qb-1. DVE order: deferred scale
                first, then recips + HOP copies (these free the pv ring slots
                the next block is about to claim), C evacs last."""
                qsl = ds(qb * 512, 512)
                pvs = blk_pvs.pop((qb, m))
                flush_scale()
                sca = scap.tile([P, 512], BF16, tag="sca")
                srows = []
                for e in range(2):
                    srow = srowp.tile([1, 512], BF16, tag="srow1")
                    with nc.allow_low_precision(reason="softmax normalizer to bf16"):
                        nc.vector.reciprocal(srow, pvs[e][64:65, :])
                    srows.append(srow)
                nc.vector.tensor_copy(HOP[0:64, m, qsl], pvs[0][0:64, :])
                nc.vector.tensor_copy(HOP[64:128, m, qsl], pvs[1][0:64, :])
                if not tail:
                    slot = 8 * qb + 2 * m
                    for e in range(2):
                        nc.gpsimd.dma_start(
                            rscr.ap()[slot + e : slot + e + 1, :], srows[e]
                        )
                    rsrc = bass.AP(
                        tensor=rscr.ap().tensor,
                        offset=slot * 512,
                        ap=[[512, 2], [0, 64], [1, 512]],
                    )
                    nc.sync.dma_start(sca, rsrc)
                    pending_scale.append((m, qsl, sca))
                if qb >= 1:
                    c_pair(qb - 1, 2 * m, tail=tail)
                if tail:
                    # latency-critical last block: broadcast the recip rows
                    # with PE matmuls instead of the DRAM round trip (the
                    # c_pair above keeps PE busy during the recip latency)
                    bc = psS.tile([P, 2, 512], F32, tag="s", name="bcast")
                    for e in range(2):
                        nc.tensor.matmul(
                            bc[e * 64 : e * 64 + 64, 0, :],
                            lhsT=ones_sb[0:1, 0:64],
                            rhs=srows[e],
                            start=True,
                            stop=True,
                        )
                    nc.vector.tensor_copy(sca, bc[:, 0, :])
                    nc.vector.tensor_mul(HOP[:, m, qsl], HOP[:, m, qsl], sca)

            # flat software pipeline over all (qb, m, kt) units: PV matmuls
            # lag the S/exp pair by two units so PE never waits on exp latency
            units = [
                (qb, m, kt)
                for qb in range(4)
                for m in range(4)
                for kt in range(4 * qb + 4)
            ]

            pipe = []

            def drain_one():
                q_, m_, k_, p_, c_ = pipe.pop(0)
                pv_unit(q_, m_, k_, p_, c_)
                if k_ == 4 * q_ + 3:
                    norm_block(q_, m_, tail=(q_ == 3 and m_ == 3))

            for qb, m, kt in units:
                pt2, c0 = s_exp_unit(qb, m, kt)
                pipe.append((qb, m, kt, pt2, c0))
                if len(pipe) > 2:
                    drain_one()
            while pipe:
                drain_one()

            # tail: last qb's phase C through the shared PSUM ring
            for ot0 in (0, 2, 4, 6):
                c_pair(3, ot0, tail=True)


# ---------------- host side ----------------

def _host_tables():
    import ml_dtypes

    i = np.arange(32, dtype=np.float32)
    inv_freq = (THETA ** (2.0 * i / DK)).astype(np.float32)
    t = np.arange(S, dtype=np.float32)
    ang = t[:, None] / inv_freq[None, :]  # [S, 32]
    c = np.cos(ang).astype(np.float32).T  # [32, S]
    sn = np.sin(ang).astype(np.float32).T
    cosP = np.tile(c, (4, 1))  # [128, S]
    sinP = np.tile(sn, (4, 1))
    sign = np.repeat(np.array([-1.0, 1.0, -1.0, 1.0], dtype=np.float32), 32)
    sinP = sinP * sign[:, None]

    kk = np.arange(P)[:, None]
    qq = np.arange(P)[None, :]
    keep = (kk <= qq).astype(ml_dtypes.bfloat16)  # [128,128]
    trimask = np.tile(keep, (1, 2))  # [128, 2*128] (both head halves)
    bf = ml_dtypes.bfloat16
    return cosP.astype(bf), sinP.astype(bf), trimask


_PERM = np.concatenate(
    [np.concatenate([h * 64 + np.arange(0, 64, 2), h * 64 + np.arange(1, 64, 2)])
     for h in range(NH)]
)

_NC_CACHE = {}


def make_in_maps(x, Wq, Wk, Wv, Wo):
    import ml_dtypes

    bf = ml_dtypes.bfloat16
    cosP, sinP, trimask = _host_tables()
    in_maps = []
    for c in range(8):
        b, hh = c // 2, c % 2
        sl = slice(hh * HD, (hh + 1) * HD)
        in_maps.append(
            {
                "xT": np.ascontiguousarray(x[b].T).astype(bf),
                "wqT": np.ascontiguousarray(Wq[sl, :][_PERM].T).astype(bf),
                "wkT": np.ascontiguousarray(Wk[sl, :][_PERM].T).astype(bf),
                "wvT": np.ascontiguousarray(Wv[sl, :].T).astype(bf),
                "woT": np.ascontiguousarray(Wo[:, sl].T).astype(bf),
                "cosP": cosP,
                "sinP": sinP,
                "trimask": trimask,
                "onesc": np.ones((P, P), dtype=bf),
            }
        )
    return in_maps


def gather_out(core_outs):
    out = np.empty((B, S, D), dtype=np.float32)
    for b in range(B):
        a = np.asarray(core_outs[2 * b]["outT"], dtype=np.float32)
        bb = np.asarray(core_outs[2 * b + 1]["outT"], dtype=np.float32)
        out[b] = (a + bb).T
    return out


def kernel(x, Wq, Wk, Wv, Wo):
    x = np.asarray(x, dtype=np.float32)
    Wq = np.asarray(Wq, dtype=np.float32)
    Wk = np.asarray(Wk, dtype=np.float32)
    Wv = np.asarray(Wv, dtype=np.float32)
    Wo = np.asarray(Wo, dtype=np.float32)

    if "nc" not in _NC_CACHE:
        _NC_CACHE["nc"] = build_attention_nc()
    nc = _NC_CACHE["nc"]

    in_maps = make_in_maps(x, Wq, Wk, Wv, Wo)
    res = run_bass_kernel_spmd(nc, in_maps, core_ids=list(range(8)))
    return gather_out(res.results)


# revision 44
# speedup vs baseline: 1.5682x; 1.0129x over previous
"""Causal multi-head attention (B=4, S=2048, D=1024, H=16, RoPE) on 8 trn2 cores.

Sharding: core c -> batch c//2, head-half c%2 (8 heads / 512 dims per core).
Each core computes QKV projections for its head slice, RoPE, causal flash
attention, and a partial output projection with its Wo column slice; the host
sums the two partials per batch (the tensor-parallel all-reduce) and
transposes back.

v2 layout/scheduling (vs the 406us baseline; timeline-sim 259us):
  - bf16 everywhere outside PSUM accumulation (inputs host-cast): same PE
    rate as fp32r but no 256-wide fp32r floor, half the DMA bytes, and 2x
    DVE throughput on the elementwise ops
  - x loaded ONCE into SBUF (32KB/partition) and reused for the V pass;
    startup DMAs split across the SP+ACT HWDGE queues in first-use order
  - pass 1 is tci-outer (matches x chunk arrival); RoPE runs per (tensor,
    j, half-row) as soon as both its chunks exist (swap via 4 partition-
    block DMAs + 3 DVE bf16 TTs), spread across pass 1; pass-1/V PSUM
    evacuations all on ACT (idle in phase A)
  - causal mask is a multiplicative 0/1 bf16 TT on the diagonal 128-block
    of pt2 AFTER the exp (DVE), so S needs one matmul per (e, kt) and PE
    carries no mask matmuls at all
  - flat software pipeline over all (qb, m, kt) units: PV matmuls lag the
    S/exp pair by two units, crossing block boundaries, so PE never waits
    on the exp latency; ACT carries exps only
  - softmax normalizer: ones-column fused in V' gives the row sum; DVE
    reciprocal (psum row -> partition-0 bf16) -> rscr DRAM round trip
    (out on Pool SWDGE, one merged 2x64-partition broadcast back on SP);
    the HOP scale-multiply is deferred one m-block; pv evac is two direct
    DVE copies (DVE handles the 0->64 partition shift for e1)
  - phase C (output projection) runs as paired column groups sharing one
    store DMA, interleaved one block AND one m-slot behind the attention
    so neither the HOP scale nor the PSUM evac ever stalls PE; PSUM is
    psS 2x2 banks + a shared 4-slot ring for pv pairs and phase-C tiles
  - last block's normalizer broadcast is done with PE matmuls (ones
    outer product) instead of the DRAM round trip to shorten the tail
"""

import numpy as np

import concourse.bass as bass
import concourse.bacc as bacc
import concourse.mybir as mybir
import concourse.tile as tile
from concourse.bass import ds, ts
from concourse.bass_utils import run_bass_kernel_spmd

F32 = mybir.dt.float32
BF16 = mybir.dt.bfloat16

B, S, D, H, DK = 4, 2048, 1024, 16, 64
THETA = 10000.0
NH = 8  # heads per core
HD = NH * DK  # 512 head dims per core
P = 128
NEG = -1.0e9
EXPF = mybir.ActivationFunctionType.Exp


def build_attention_nc(nrep=1):
    nc = bacc.Bacc("TRN2", target_bir_lowering=False, debug=False)

    xT = nc.dram_tensor("xT", [D, S], BF16, kind="ExternalInput")
    wqT = nc.dram_tensor("wqT", [D, HD], BF16, kind="ExternalInput")
    wkT = nc.dram_tensor("wkT", [D, HD], BF16, kind="ExternalInput")
    wvT = nc.dram_tensor("wvT", [D, HD], BF16, kind="ExternalInput")
    woT = nc.dram_tensor("woT", [HD, D], BF16, kind="ExternalInput")
    cosP = nc.dram_tensor("cosP", [P, S], BF16, kind="ExternalInput")
    sinP = nc.dram_tensor("sinP", [P, S], BF16, kind="ExternalInput")
    trimask = nc.dram_tensor("trimask", [P, 2 * P], BF16, kind="ExternalInput")
    onesc = nc.dram_tensor("onesc", [P, P], BF16, kind="ExternalInput")
    outT = nc.dram_tensor("outT", [D, S], BF16, kind="ExternalOutput")
    rscr = nc.dram_tensor("rscr", [NH * 4, 512], BF16, kind="Internal")

    with tile.TileContext(nc) as tc:
        if nrep == 1:
            _attention_tile(
                tc, xT, wqT, wkT, wvT, woT, cosP, sinP, trimask, onesc,
                outT, rscr,
            )
        else:
            with tc.For_i(0, nrep, 1):
                _attention_tile(
                    tc, xT, wqT, wkT, wvT, woT, cosP, sinP, trimask,
                    onesc, outT, rscr,
                )
    nc.compile()
    return nc


def _attention_tile(tc, xT, wqT, wkT, wvT, woT, cosP, sinP, trimask, onesc, outT, rscr):
    nc = tc.nc

    with tc.tile_pool(name="qkv", bufs=1) as qkv:
        # ---- persistent tiles ----
        x_sb = qkv.tile([P, 8, S], BF16, tag="x")     # [k%128, k//128, t]
        QT = qkv.tile([P, 4, S], BF16, tag="QT")      # [d'%128, d'//128, t]
        KT = qkv.tile([P, 4, S], BF16, tag="KT")
        VP = qkv.tile([P, 16, 528], BF16, tag="VP")   # [t%128, t//128, 8*(64+ones+pad)]
        HOP = qkv.tile([P, 4, S], BF16, tag="HOP")    # head pairs x [128 dv, t]
        cos_sb = qkv.tile([P, S], BF16, tag="cos")
        sin_sb = qkv.tile([P, S], BF16, tag="sin")
        wq_sb = qkv.tile([P, 8, HD], BF16, tag="wq")
        wk_sb = qkv.tile([P, 8, HD], BF16, tag="wk")
        wv_sb = qkv.tile([P, 8, HD], BF16, tag="wv")
        wo_sb = qkv.tile([P, 4, D], BF16, tag="wo")
        tri_sb = qkv.tile([P, 2, P], BF16, tag="tri")

        ones_sb = qkv.tile([1, P], BF16, tag="ones1")

        # ---- bulk input DMAs, all up front on the SP HWDGE queue, in
        # first-use order (x chunk 0 and wq gate the first matmul) ----
        xT_t = xT.ap().rearrange("(o p) t -> p o t", p=P)
        wq_src = wqT.ap().rearrange("(o p) d -> p o d", p=P)
        # first mm group needs x chunk 0 + wq; split both so the halves land
        # (and the first 4-step accumulation starts) as early as possible
        # wq pieces ride the ACT HWDGE queue, x pieces the SP queue, so the
        # two issue pipelines overlap and the first 2-block accumulation can
        # start as soon as its operands land
        nc.sync.dma_start(wq_sb[:, 0:2, :], wq_src[:, 0:2, :])
        nc.scalar.dma_start(x_sb[:, 0:2, ds(0, 512)], xT_t[:, 0:2, ds(0, 512)])
        nc.sync.dma_start(wq_sb[:, 2:4, :], wq_src[:, 2:4, :])
        nc.scalar.dma_start(x_sb[:, 2:4, ds(0, 512)], xT_t[:, 2:4, ds(0, 512)])
        nc.sync.dma_start(wq_sb[:, 4:8, :], wq_src[:, 4:8, :])
        nc.scalar.dma_start(x_sb[:, 4:8, ds(0, 512)], xT_t[:, 4:8, ds(0, 512)])
        nc.scalar.dma_start(wk_sb, wkT.ap().rearrange("(o p) d -> p o d", p=P))
        for tci in range(1, 4):
            nc.sync.dma_start(x_sb[:, :, ds(tci * 512, 512)], xT_t[:, :, ds(tci * 512, 512)])
        nc.sync.dma_start(cos_sb, cosP.ap())
        nc.sync.dma_start(sin_sb, sinP.ap())
        nc.sync.dma_start(tri_sb, trimask.ap().rearrange("p (e q) -> p e q", e=2))
        nc.sync.dma_start(ones_sb, onesc.ap()[0:1, :])
        # ones columns of V' (col 64 of each 66-wide head group)
        vp_g = VP[:, :, :].rearrange("p k (h c) -> p k h c", c=66)
        nc.sync.dma_start(
            vp_g[:, :, :, 64:65],
            onesc.ap().rearrange("p (k h one) -> p k h one", k=16, one=1),
        )
        nc.sync.dma_start(wo_sb, woT.ap().rearrange("(m p) o -> p m o", p=P))
        nc.sync.dma_start(wv_sb, wvT.ap().rearrange("(o p) d -> p o d", p=P))

        # ---- phase A: Q+K (tci-outer matches x arrival; RoPE per (dst,j)
        # once its last chunk lands), then V ----
        with (
            tc.tile_pool(name="swpool", bufs=2) as swpool,
            tc.tile_pool(name="psA", bufs=6, space="PSUM") as psA,
        ):
            for tci in range(4):
                for dst, w_sb in ((QT, wq_sb), (KT, wk_sb)):
                    for j in range(4):
                        ps = psA.tile([P, 512], F32, tag="psA")
                        for i in range(8):
                            nc.tensor.matmul(
                                ps,
                                lhsT=w_sb[:, i, ts(j, P)],
                                rhs=x_sb[:, i, ds(tci * 512, 512)],
                                start=(i == 0),
                                stop=(i == 7),
                            )
                        nc.scalar.copy(dst[:, j, ds(tci * 512, 512)], ps)
                        if tci % 2 == 1:
                            # RoPE for this (tensor, j) half-row as soon as
                            # both its chunks exist: pair-swap via 4
                            # partition-block DMAs (Q on the SP queue, K on
                            # the ACT queue so neither backs up), 3 DVE bf16
                            # TTs. Half-row granularity spreads the DVE work
                            # across pass 1 instead of piling it at the end.
                            hsl = ds((tci // 2) * 1024, 1024)
                            qsw = swpool.tile([P, 1024], BF16, tag="qsw")
                            for blk in range(4):
                                sb = blk + (1 if blk % 2 == 0 else -1)
                                nc.sync.dma_start(
                                    qsw[blk * 32 : blk * 32 + 32, :],
                                    dst[sb * 32 : sb * 32 + 32, j, hsl],
                                )
                            tmp = swpool.tile([P, 1024], BF16, tag="rtmp")
                            nc.vector.tensor_mul(tmp, cos_sb[:, hsl], dst[:, j, hsl])
                            nc.vector.tensor_mul(qsw, sin_sb[:, hsl], qsw)
                            nc.vector.tensor_add(dst[:, j, hsl], tmp, qsw)

            # V pass (x already resident)
            for tci in range(4):
                for tt in range(4):
                    ps = psA.tile([P, 512], F32, tag="psA")
                    for i in range(8):
                        nc.tensor.matmul(
                            ps,
                            lhsT=x_sb[:, i, ds(tci * 512 + tt * P, P)],
                            rhs=wv_sb[:, i, :],
                            start=(i == 0),
                            stop=(i == 7),
                        )
                    kt_idx = tci * 4 + tt
                    nc.scalar.copy(
                        vp_g[:, kt_idx, :, 0:64],
                        ps.rearrange("p (h c) -> p h c", c=64),
                    )

        # ---- phase B: attention per (qb, m) + interleaved phase C ----
        with (
            tc.tile_pool(name="ptile", bufs=4) as ptile,
            tc.tile_pool(name="srowp", bufs=4) as srowp,
            tc.tile_pool(name="scap", bufs=2) as scap,
            tc.tile_pool(name="obpool", bufs=3) as obpool,
            tc.tile_pool(name="psS", bufs=2, space="PSUM") as psS,
            tc.tile_pool(name="psPV", bufs=4, space="PSUM") as psPV,
        ):
            psC = psPV  # phase-C groups share the 4-slot ring
            outT_ap = outT.ap()
            pending_scale = []  # deferred HOP scale-mul: (m, qsl, sca)

            def flush_scale():
                while pending_scale:
                    pm, pqsl, psca = pending_scale.pop(0)
                    nc.vector.tensor_mul(HOP[:, pm, pqsl], HOP[:, pm, pqsl], psca)

            outT_t = outT_ap.rearrange("(o p) t -> p o t", p=P)

            def c_pair(cqb, ot0, tail=False):
                """Two output-projection column groups sharing one ob tile
                and one paired outT store (halves the store DMA count)."""
                ob2 = obpool.tile([P, 2, 512], BF16, tag="ob")
                for k in range(2):
                    ot = ot0 + k
                    ps = psC.tile([P, 512], F32, tag="pv", name=f"c{cqb}_{ot}")
                    for mm in range(4):
                        nc.tensor.matmul(
                            ps,
                            lhsT=wo_sb[:, mm, ts(ot, P)],
                            rhs=HOP[:, mm, ds(cqb * 512, 512)],
                            start=(mm == 0),
                            stop=(mm == 3),
                        )
                    # at the tail split evacs over ACT+DVE so the final
                    # drain pipelines
                    if tail and k:
                        nc.scalar.copy(ob2[:, k, :], ps)
                    else:
                        nc.vector.tensor_copy(ob2[:, k, :], ps)
                dma_eng = nc.sync if tail and (ot0 // 2) % 2 else nc.gpsimd
                dma_eng.dma_start(
                    outT_t[:, ot0 : ot0 + 2, ds(cqb * 512, 512)], ob2
                )

            def s_exp_unit(qb, m, kt):
                """S matmuls + exp for one (qb, m, kt) unit; returns PV args."""
                roff = kt - 4 * qb
                c0 = max(0, 128 * roff)
                qsl = ds(qb * 512, 512)
                s2 = psS.tile([P, 2, 512], F32, tag="s")
                for e in range(2):
                    rb = e * 64
                    nc.tensor.matmul(
                        s2[:, e, c0:],
                        lhsT=KT[rb : rb + 64, m, ts(kt, P)],
                        rhs=QT[rb : rb + 64, m, ds(qb * 512 + c0, 512 - c0)],
                        start=True,
                        stop=True,
                    )
                pt2 = ptile.tile([P, 2, 512], BF16, tag="pt")
                nc.scalar.activation(pt2[:, :, c0:], s2[:, :, c0:], EXPF, scale=0.125)
                if roff >= 0:
                    # causal mask: zero the upper triangle of the diagonal
                    # 128-block multiplicatively post-exp. GPSIMD, not DVE:
                    # the DVE queue carries the norm chains, whose head-of-
                    # line waits would delay the mask and stall the PV
                    nc.vector.tensor_mul(
                        pt2[:, :, ds(c0, P)], pt2[:, :, ds(c0, P)], tri_sb
                    )
                return pt2, c0

            blk_pvs = {}  # (qb, m) -> pv tile pair

            def pv_unit(qb, m, kt, pt2, c0):
                nkt = 4 * qb + 4
                if kt == 0:
                    blk_pvs[(qb, m)] = [
                        psPV.tile([P, 512], F32, tag="pv", name=f"pv{qb}{m}{e}")
                        for e in range(2)
                    ]
                pvs = blk_pvs[(qb, m)]
                for e in range(2):
                    nc.tensor.matmul(
                        pvs[e][0:65, c0:],
                        lhsT=VP[:, kt, ds((2 * m + e) * 66, 65)],
                        rhs=pt2[:, e, c0:],
                        start=(kt == 0),
                        stop=(kt == nkt - 1),
                    )

            def norm_block(qb, m, tail=False):
                """Normalizer + pv evac for a finished (qb, m) block, then the
                scheduled phase-C pair of qb-1. DVE order: deferred scale
                first, then recips + HOP copies (these free the pv ring slots
                the next block is about to claim), C evacs last."""
                qsl = ds(qb * 512, 512)
                pvs = blk_pvs.pop((qb, m))
                flush_scale()
                sca = scap.tile([P, 512], BF16, tag="sca")
                srows = []
                for e in range(2):
                    srow = srowp.tile([1, 512], BF16, tag="srow1")
                    with nc.allow_low_precision(reason="softmax normalizer to bf16"):
                        nc.vector.reciprocal(srow, pvs[e][64:65, :])
                    srows.append(srow)
                nc.vector.tensor_copy(HOP[0:64, m, qsl], pvs[0][0:64, :])
                nc.vector.tensor_copy(HOP[64:128, m, qsl], pvs[1][0:64, :])
                if not tail:
                    slot = 8 * qb + 2 * m
                    for e in range(2):
                        nc.gpsimd.dma_start(
                            rscr.ap()[slot + e : slot + e + 1, :], srows[e]
                        )
                    rsrc = bass.AP(
                        tensor=rscr.ap().tensor,
                        offset=slot * 512,
                        ap=[[512, 2], [0, 64], [1, 512]],
                    )
                    nc.sync.dma_start(sca, rsrc)
                    pending_scale.append((m, qsl, sca))
                if qb >= 1 and m >= 1:
                    c_pair(qb - 1, 2 * (m - 1), tail=tail)
                elif qb >= 2 and m == 0:
                    c_pair(qb - 2, 6, tail=tail)
                if tail:
                    # latency-critical last block: broadcast the recip rows
                    # with PE matmuls instead of the DRAM round trip (the
                    # c_pair above keeps PE busy during the recip latency)
                    bc = psS.tile([P, 2, 512], F32, tag="s", name="bcast")
                    for e in range(2):
                        nc.tensor.matmul(
                            bc[e * 64 : e * 64 + 64, 0, :],
                            lhsT=ones_sb[0:1, 0:64],
                            rhs=srows[e],
                            start=True,
                            stop=True,
                        )
                    nc.vector.tensor_copy(sca, bc[:, 0, :])
                    nc.vector.tensor_mul(HOP[:, m, qsl], HOP[:, m, qsl], sca)

            # flat software pipeline over all (qb, m, kt) units: PV matmuls
            # lag the S/exp pair by two units so PE never waits on exp latency
            units = [
                (qb, m, kt)
                for qb in range(4)
                for m in range(4)
                for kt in range(4 * qb + 4)
            ]

            pipe = []

            def drain_one():
                q_, m_, k_, p_, c_ = pipe.pop(0)
                pv_unit(q_, m_, k_, p_, c_)
                if k_ == 4 * q_ + 3:
                    norm_block(q_, m_, tail=(q_ == 3 and m_ == 3))

            for qb, m, kt in units:
                pt2, c0 = s_exp_unit(qb, m, kt)
                pipe.append((qb, m, kt, pt2, c0))
                if len(pipe) > 2:
                    drain_one()
            while pipe:
                drain_one()

            # tail: the carried pair plus last qb's phase C
            c_pair(2, 6, tail=True)
            for ot0 in (0, 2, 4, 6):
                c_pair(3, ot0, tail=True)


# ---------------- host side ----------------

def _host_tables():
    import ml_dtypes

    i = np.arange(32, dtype=np.float32)
    inv_freq = (THETA ** (2.0 * i / DK)).astype(np.float32)
    t = np.arange(S, dtype=np.float32)
    ang = t[:, None] / inv_freq[None, :]  # [S, 32]
    c = np.cos(ang).astype(np.float32).T  # [32, S]
    sn = np.sin(ang).astype(np.float32).T
    cosP = np.tile(c, (4, 1))  # [128, S]
    sinP = np.tile(sn, (4, 1))
    sign = np.repeat(np.array([-1.0, 1.0, -1.0, 1.0], dtype=np.float32), 32)
    sinP = sinP * sign[:, None]

    kk = np.arange(P)[:, None]
    qq = np.arange(P)[None, :]
    keep = (kk <= qq).astype(ml_dtypes.bfloat16)  # [128,128]
    trimask = np.tile(keep, (1, 2))  # [128, 2*128] (both head halves)
    bf = ml_dtypes.bfloat16
    return cosP.astype(bf), sinP.astype(bf), trimask


_PERM = np.concatenate(
    [np.concatenate([h * 64 + np.arange(0, 64, 2), h * 64 + np.arange(1, 64, 2)])
     for h in range(NH)]
)

_NC_CACHE = {}


def make_in_maps(x, Wq, Wk, Wv, Wo):
    import ml_dtypes

    bf = ml_dtypes.bfloat16
    cosP, sinP, trimask = _host_tables()
    in_maps = []
    for c in range(8):
        b, hh = c // 2, c % 2
        sl = slice(hh * HD, (hh + 1) * HD)
        in_maps.append(
            {
                "xT": np.ascontiguousarray(x[b].T).astype(bf),
                "wqT": np.ascontiguousarray(Wq[sl, :][_PERM].T).astype(bf),
                "wkT": np.ascontiguousarray(Wk[sl, :][_PERM].T).astype(bf),
                "wvT": np.ascontiguousarray(Wv[sl, :].T).astype(bf),
                "woT": np.ascontiguousarray(Wo[:, sl].T).astype(bf),
                "cosP": cosP,
                "sinP": sinP,
                "trimask": trimask,
                "onesc": np.ones((P, P), dtype=bf),
            }
        )
    return in_maps


def gather_out(core_outs):
    out = np.empty((B, S, D), dtype=np.float32)
    for b in range(B):
        a = np.asarray(core_outs[2 * b]["outT"], dtype=np.float32)
        bb = np.asarray(core_outs[2 * b + 1]["outT"], dtype=np.float32)
        out[b] = (a + bb).T
    return out


def kernel(x, Wq, Wk, Wv, Wo):
    x = np.asarray(x, dtype=np.float32)
    Wq = np.asarray(Wq, dtype=np.float32)
    Wk = np.asarray(Wk, dtype=np.float32)
    Wv = np.asarray(Wv, dtype=np.float32)
    Wo = np.asarray(Wo, dtype=np.float32)

    if "nc" not in _NC_CACHE:
        _NC_CACHE["nc"] = build_attention_nc()
    nc = _NC_CACHE["nc"]

    in_maps = make_in_maps(x, Wq, Wk, Wv, Wo)
    res = run_bass_kernel_spmd(nc, in_maps, core_ids=list(range(8)))
    return gather_out(res.results)


# revision 50
# speedup vs baseline: 1.5826x; 1.0091x over previous
"""Causal multi-head attention (B=4, S=2048, D=1024, H=16, RoPE) on 8 trn2 cores.

Sharding: core c -> batch c//2, head-half c%2 (8 heads / 512 dims per core).
Each core computes QKV projections for its head slice, RoPE, causal flash
attention, and a partial output projection with its Wo column slice; the host
sums the two partials per batch (the tensor-parallel all-reduce) and
transposes back.

v2 layout/scheduling (vs the 406us baseline; timeline-sim 259us):
  - bf16 everywhere outside PSUM accumulation (inputs host-cast): same PE
    rate as fp32r but no 256-wide fp32r floor, half the DMA bytes, and 2x
    DVE throughput on the elementwise ops
  - x loaded ONCE into SBUF (32KB/partition) and reused for the V pass;
    startup DMAs split across the SP+ACT HWDGE queues in first-use order
  - pass 1 is tci-outer (matches x chunk arrival); RoPE runs per (tensor,
    j, half-row) as soon as both its chunks exist (swap via 4 partition-
    block DMAs + 3 DVE bf16 TTs), spread across pass 1; pass-1/V PSUM
    evacuations all on ACT (idle in phase A)
  - causal mask is a multiplicative 0/1 bf16 TT on the diagonal 128-block
    of pt2 AFTER the exp (DVE), so S needs one matmul per (e, kt) and PE
    carries no mask matmuls at all
  - flat software pipeline over all (qb, m, kt) units: PV matmuls lag the
    S/exp pair by two units, crossing block boundaries, so PE never waits
    on the exp latency; ACT carries exps only
  - softmax normalizer: ones-column fused in V' gives the row sum; DVE
    reciprocal (psum row -> partition-0 bf16) -> rscr DRAM round trip
    (out on Pool SWDGE, one merged 2x64-partition broadcast back on SP);
    the HOP scale-multiply is deferred one m-block; pv evac is two direct
    DVE copies (DVE handles the 0->64 partition shift for e1)
  - phase C (output projection) runs as paired column groups sharing one
    store DMA, interleaved one block AND one m-slot behind the attention
    so neither the HOP scale nor the PSUM evac ever stalls PE; PSUM is
    psS 2x2 banks + a shared 4-slot ring for pv pairs and phase-C tiles
  - last block's normalizer broadcast is done with PE matmuls (ones
    outer product) instead of the DRAM round trip to shorten the tail
"""

import numpy as np

import concourse.bass as bass
import concourse.bacc as bacc
import concourse.mybir as mybir
import concourse.tile as tile
from concourse.bass import ds, ts
from concourse.bass_utils import run_bass_kernel_spmd

F32 = mybir.dt.float32
BF16 = mybir.dt.bfloat16

B, S, D, H, DK = 4, 2048, 1024, 16, 64
THETA = 10000.0
NH = 8  # heads per core
HD = NH * DK  # 512 head dims per core
P = 128
NEG = -1.0e9
EXPF = mybir.ActivationFunctionType.Exp


def build_attention_nc(nrep=1):
    nc = bacc.Bacc("TRN2", target_bir_lowering=False, debug=False)

    xT = nc.dram_tensor("xT", [D, S], BF16, kind="ExternalInput")
    wqT = nc.dram_tensor("wqT", [D, HD], BF16, kind="ExternalInput")
    wkT = nc.dram_tensor("wkT", [D, HD], BF16, kind="ExternalInput")
    wvT = nc.dram_tensor("wvT", [D, HD], BF16, kind="ExternalInput")
    woT = nc.dram_tensor("woT", [HD, D], BF16, kind="ExternalInput")
    cosP = nc.dram_tensor("cosP", [P, S], BF16, kind="ExternalInput")
    sinP = nc.dram_tensor("sinP", [P, S], BF16, kind="ExternalInput")
    trimask = nc.dram_tensor("trimask", [P, 2 * P], BF16, kind="ExternalInput")
    onesc = nc.dram_tensor("onesc", [P, P], BF16, kind="ExternalInput")
    outT = nc.dram_tensor("outT", [D, S], BF16, kind="ExternalOutput")
    rscr = nc.dram_tensor("rscr", [NH * 4, 512], BF16, kind="Internal")

    with tile.TileContext(nc) as tc:
        if nrep == 1:
            _attention_tile(
                tc, xT, wqT, wkT, wvT, woT, cosP, sinP, trimask, onesc,
                outT, rscr,
            )
        else:
            with tc.For_i(0, nrep, 1):
                _attention_tile(
                    tc, xT, wqT, wkT, wvT, woT, cosP, sinP, trimask,
                    onesc, outT, rscr,
                )
    nc.compile()
    return nc


def _attention_tile(tc, xT, wqT, wkT, wvT, woT, cosP, sinP, trimask, onesc, outT, rscr):
    nc = tc.nc

    with tc.tile_pool(name="qkv", bufs=1) as qkv:
        # ---- persistent tiles ----
        x_sb = qkv.tile([P, 8, S], BF16, tag="x")     # [k%128, k//128, t]
        QT = qkv.tile([P, 4, S], BF16, tag="QT")      # [d'%128, d'//128, t]
        KT = qkv.tile([P, 4, S], BF16, tag="KT")
        VP = qkv.tile([P, 16, 528], BF16, tag="VP")   # [t%128, t//128, 8*(64+ones+pad)]
        HOP = qkv.tile([P, 4, S], BF16, tag="HOP")    # head pairs x [128 dv, t]
        cos_sb = qkv.tile([P, S], BF16, tag="cos")
        sin_sb = qkv.tile([P, S], BF16, tag="sin")
        wq_sb = qkv.tile([P, 8, HD], BF16, tag="wq")
        wk_sb = qkv.tile([P, 8, HD], BF16, tag="wk")
        wv_sb = qkv.tile([P, 8, HD], BF16, tag="wv")
        wo_sb = qkv.tile([P, 4, D], BF16, tag="wo")
        tri_sb = qkv.tile([P, 2, P], BF16, tag="tri")

        ones_sb = qkv.tile([1, P], BF16, tag="ones1")

        # ---- bulk input DMAs, all up front on the SP HWDGE queue, in
        # first-use order (x chunk 0 and wq gate the first matmul) ----
        xT_t = xT.ap().rearrange("(o p) t -> p o t", p=P)
        wq_src = wqT.ap().rearrange("(o p) d -> p o d", p=P)
        # first mm group needs x chunk 0 + wq; split both so the halves land
        # (and the first 4-step accumulation starts) as early as possible
        # wq pieces ride the ACT HWDGE queue, x pieces the SP queue, so the
        # two issue pipelines overlap and the first 2-block accumulation can
        # start as soon as its operands land
        nc.sync.dma_start(wq_sb[:, 0:1, :], wq_src[:, 0:1, :])
        nc.scalar.dma_start(x_sb[:, 0:1, ds(0, 512)], xT_t[:, 0:1, ds(0, 512)])
        nc.sync.dma_start(wq_sb[:, 1:2, :], wq_src[:, 1:2, :])
        nc.scalar.dma_start(x_sb[:, 1:2, ds(0, 512)], xT_t[:, 1:2, ds(0, 512)])
        nc.sync.dma_start(wq_sb[:, 2:4, :], wq_src[:, 2:4, :])
        nc.scalar.dma_start(x_sb[:, 2:4, ds(0, 512)], xT_t[:, 2:4, ds(0, 512)])
        nc.sync.dma_start(wq_sb[:, 4:8, :], wq_src[:, 4:8, :])
        nc.scalar.dma_start(x_sb[:, 4:8, ds(0, 512)], xT_t[:, 4:8, ds(0, 512)])
        nc.scalar.dma_start(wk_sb, wkT.ap().rearrange("(o p) d -> p o d", p=P))
        for tci in range(1, 4):
            nc.sync.dma_start(x_sb[:, :, ds(tci * 512, 512)], xT_t[:, :, ds(tci * 512, 512)])
        nc.sync.dma_start(cos_sb, cosP.ap())
        nc.sync.dma_start(sin_sb, sinP.ap())
        nc.sync.dma_start(tri_sb, trimask.ap().rearrange("p (e q) -> p e q", e=2))
        nc.sync.dma_start(ones_sb, onesc.ap()[0:1, :])
        # ones columns of V' (col 64 of each 66-wide head group)
        vp_g = VP[:, :, :].rearrange("p k (h c) -> p k h c", c=66)
        nc.sync.dma_start(
            vp_g[:, :, :, 64:65],
            onesc.ap().rearrange("p (k h one) -> p k h one", k=16, one=1),
        )
        nc.sync.dma_start(wo_sb, woT.ap().rearrange("(m p) o -> p m o", p=P))
        nc.sync.dma_start(wv_sb, wvT.ap().rearrange("(o p) d -> p o d", p=P))

        # pt2 tiles live across phase A (pipeline warm-up) and phase B
        ptile_cm = tc.tile_pool(name="ptile", bufs=8)
        ptile = ptile_cm.__enter__()

        # ---- phase A: Q+K (tci-outer matches x arrival; RoPE per (dst,j)
        # once its last chunk lands), then V ----
        with (
            tc.tile_pool(name="swpool", bufs=2) as swpool,
            tc.tile_pool(name="psA", bufs=4, space="PSUM") as psA,
            tc.tile_pool(name="psW", bufs=2, space="PSUM") as psW,
        ):
            for tci in range(4):
                for dst, w_sb in ((QT, wq_sb), (KT, wk_sb)):
                    for j in range(4):
                        ps = psA.tile([P, 512], F32, tag="psA")
                        for i in range(8):
                            nc.tensor.matmul(
                                ps,
                                lhsT=w_sb[:, i, ts(j, P)],
                                rhs=x_sb[:, i, ds(tci * 512, 512)],
                                start=(i == 0),
                                stop=(i == 7),
                            )
                        nc.scalar.copy(dst[:, j, ds(tci * 512, 512)], ps)
                        if tci % 2 == 1:
                            # RoPE for this (tensor, j) half-row as soon as
                            # both its chunks exist: pair-swap via 4
                            # partition-block DMAs (Q on the SP queue, K on
                            # the ACT queue so neither backs up), 3 DVE bf16
                            # TTs. Half-row granularity spreads the DVE work
                            # across pass 1 instead of piling it at the end.
                            hsl = ds((tci // 2) * 1024, 1024)
                            qsw = swpool.tile([P, 1024], BF16, tag="qsw")
                            for blk in range(4):
                                sb = blk + (1 if blk % 2 == 0 else -1)
                                nc.sync.dma_start(
                                    qsw[blk * 32 : blk * 32 + 32, :],
                                    dst[sb * 32 : sb * 32 + 32, j, hsl],
                                )
                            tmp = swpool.tile([P, 1024], BF16, tag="rtmp")
                            nc.vector.tensor_mul(tmp, cos_sb[:, hsl], dst[:, j, hsl])
                            nc.vector.tensor_mul(qsw, sin_sb[:, hsl], qsw)
                            nc.vector.tensor_add(dst[:, j, hsl], tmp, qsw)

            # V pass (x already resident), with the first attention
            # block's S+exp units woven in on spare PSUM banks: the exp
            # chain on ACT is warmed up before phase B even starts
            warm = []

            def warm_unit(kt, wm=0):
                c0 = 128 * kt
                s2 = psW.tile([P, 2, 512], F32, tag="s2w")
                for e in range(2):
                    rb = e * 64
                    nc.tensor.matmul(
                        s2[:, e, c0:],
                        lhsT=KT[rb : rb + 64, wm, ts(kt, P)],
                        rhs=QT[rb : rb + 64, wm, ds(c0, 512 - c0)],
                        start=True,
                        stop=True,
                    )
                pt2 = ptile.tile([P, 2, 512], BF16, tag="pt")
                nc.scalar.activation(pt2[:, :, c0:], s2[:, :, c0:], EXPF, scale=0.125)
                nc.vector.tensor_mul(
                    pt2[:, :, ds(c0, P)], pt2[:, :, ds(c0, P)], tri_sb
                )
                warm.append((0, wm, kt, pt2, c0))

            for tci in range(4):
                for tt in range(4):
                    ps = psA.tile([P, 512], F32, tag="psA")
                    for i in range(8):
                        nc.tensor.matmul(
                            ps,
                            lhsT=x_sb[:, i, ds(tci * 512 + tt * P, P)],
                            rhs=wv_sb[:, i, :],
                            start=(i == 0),
                            stop=(i == 7),
                        )
                    kt_idx = tci * 4 + tt
                    nc.scalar.copy(
                        vp_g[:, kt_idx, :, 0:64],
                        ps.rearrange("p (h c) -> p h c", c=64),
                    )
                    if kt_idx >= 6 and kt_idx % 2 == 0:
                        u = (kt_idx - 6) // 2
                        warm_unit(u % 4, wm=u // 4)
                    if kt_idx == 15:
                        for u in (5, 6, 7):
                            warm_unit(u % 4, wm=u // 4)

        # ---- phase B: attention per (qb, m) + interleaved phase C ----
        with (
            tc.tile_pool(name="srowp", bufs=4) as srowp,
            tc.tile_pool(name="scap", bufs=2) as scap,
            tc.tile_pool(name="obpool", bufs=3) as obpool,
            tc.tile_pool(name="psS", bufs=2, space="PSUM") as psS,
            tc.tile_pool(name="psPV", bufs=4, space="PSUM") as psPV,
        ):
            psC = psPV  # phase-C groups share the 4-slot ring
            outT_ap = outT.ap()
            pending_scale = []  # deferred HOP scale-mul: (m, qsl, sca)

            def flush_scale():
                while pending_scale:
                    pm, pqsl, psca = pending_scale.pop(0)
                    nc.vector.tensor_mul(HOP[:, pm, pqsl], HOP[:, pm, pqsl], psca)

            outT_t = outT_ap.rearrange("(o p) t -> p o t", p=P)

            def c_pair(cqb, ot0, tail=False):
                """Two output-projection column groups sharing one ob tile
                and one paired outT store (halves the store DMA count)."""
                ob2 = obpool.tile([P, 2, 512], BF16, tag="ob")
                for k in range(2):
                    ot = ot0 + k
                    ps = psC.tile([P, 512], F32, tag="pv", name=f"c{cqb}_{ot}")
                    for mm in range(4):
                        nc.tensor.matmul(
                            ps,
                            lhsT=wo_sb[:, mm, ts(ot, P)],
                            rhs=HOP[:, mm, ds(cqb * 512, 512)],
                            start=(mm == 0),
                            stop=(mm == 3),
                        )
                    # at the tail split evacs over ACT+DVE so the final
                    # drain pipelines
                    if tail and k:
                        nc.scalar.copy(ob2[:, k, :], ps)
                    else:
                        nc.vector.tensor_copy(ob2[:, k, :], ps)
                dma_eng = nc.sync if tail and (ot0 // 2) % 2 else nc.gpsimd
                dma_eng.dma_start(
                    outT_t[:, ot0 : ot0 + 2, ds(cqb * 512, 512)], ob2
                )

            def s_exp_unit(qb, m, kt):
                """S matmuls + exp for one (qb, m, kt) unit; returns PV args."""
                roff = kt - 4 * qb
                c0 = max(0, 128 * roff)
                qsl = ds(qb * 512, 512)
                s2 = psS.tile([P, 2, 512], F32, tag="s")
                for e in range(2):
                    rb = e * 64
                    nc.tensor.matmul(
                        s2[:, e, c0:],
                        lhsT=KT[rb : rb + 64, m, ts(kt, P)],
                        rhs=QT[rb : rb + 64, m, ds(qb * 512 + c0, 512 - c0)],
                        start=True,
                        stop=True,
                    )
                pt2 = ptile.tile([P, 2, 512], BF16, tag="pt")
                nc.scalar.activation(pt2[:, :, c0:], s2[:, :, c0:], EXPF, scale=0.125)
                if roff >= 0:
                    # causal mask: zero the upper triangle of the diagonal
                    # 128-block multiplicatively post-exp. GPSIMD, not DVE:
                    # the DVE queue carries the norm chains, whose head-of-
                    # line waits would delay the mask and stall the PV
                    nc.vector.tensor_mul(
                        pt2[:, :, ds(c0, P)], pt2[:, :, ds(c0, P)], tri_sb
                    )
                return pt2, c0

            blk_pvs = {}  # (qb, m) -> pv tile pair

            def pv_unit(qb, m, kt, pt2, c0):
                nkt = 4 * qb + 4
                if kt == 0:
                    blk_pvs[(qb, m)] = [
                        psPV.tile([P, 512], F32, tag="pv", name=f"pv{qb}{m}{e}")
                        for e in range(2)
                    ]
                pvs = blk_pvs[(qb, m)]
                for e in range(2):
                    nc.tensor.matmul(
                        pvs[e][0:65, c0:],
                        lhsT=VP[:, kt, ds((2 * m + e) * 66, 65)],
                        rhs=pt2[:, e, c0:],
                        start=(kt == 0),
                        stop=(kt == nkt - 1),
                    )

            def norm_block(qb, m, tail=False):
                """Normalizer + pv evac for a finished (qb, m) block, then the
                scheduled phase-C pair of qb-1. DVE order: deferred scale
                first, then recips + HOP copies (these free the pv ring slots
                the next block is about to claim), C evacs last."""
                qsl = ds(qb * 512, 512)
                pvs = blk_pvs.pop((qb, m))
                flush_scale()
                sca = scap.tile([P, 512], BF16, tag="sca")
                srows = []
                for e in range(2):
                    srow = srowp.tile([1, 512], BF16, tag="srow1")
                    with nc.allow_low_precision(reason="softmax normalizer to bf16"):
                        nc.vector.reciprocal(srow, pvs[e][64:65, :])
                    srows.append(srow)
                nc.vector.tensor_copy(HOP[0:64, m, qsl], pvs[0][0:64, :])
                nc.vector.tensor_copy(HOP[64:128, m, qsl], pvs[1][0:64, :])
                if not tail:
                    slot = 8 * qb + 2 * m
                    for e in range(2):
                        nc.gpsimd.dma_start(
                            rscr.ap()[slot + e : slot + e + 1, :], srows[e]
                        )
                    rsrc = bass.AP(
                        tensor=rscr.ap().tensor,
                        offset=slot * 512,
                        ap=[[512, 2], [0, 64], [1, 512]],
                    )
                    nc.sync.dma_start(sca, rsrc)
                    pending_scale.append((m, qsl, sca))
                if qb >= 1 and m >= 1:
                    c_pair(qb - 1, 2 * (m - 1), tail=tail)
                elif qb >= 2 and m == 0:
                    c_pair(qb - 2, 6, tail=tail)
                if tail:
                    # latency-critical last block: broadcast the recip rows
                    # with PE matmuls instead of the DRAM round trip (the
                    # c_pair above keeps PE busy during the recip latency)
                    bc = psS.tile([P, 2, 512], F32, tag="s", name="bcast")
                    for e in range(2):
                        nc.tensor.matmul(
                            bc[e * 64 : e * 64 + 64, 0, :],
                            lhsT=ones_sb[0:1, 0:64],
                            rhs=srows[e],
                            start=True,
                            stop=True,
                        )
                    nc.vector.tensor_copy(sca, bc[:, 0, :])
                    nc.vector.tensor_mul(HOP[:, m, qsl], HOP[:, m, qsl], sca)

            # flat software pipeline over all (qb, m, kt) units: PV matmuls
            # lag the S/exp pair by two units so PE never waits on exp latency
            units = [
                (qb, m, kt)
                for qb in range(4)
                for m in range(4)
                for kt in range(4 * qb + 4)
            ][8:]  # (0, m0) and (0, m1) warmed up during the V pass

            pipe = list(warm)

            def drain_one():
                q_, m_, k_, p_, c_ = pipe.pop(0)
                pv_unit(q_, m_, k_, p_, c_)
                if k_ == 4 * q_ + 3:
                    norm_block(q_, m_, tail=(q_ == 3 and m_ == 3))

            while len(pipe) > 2:
                drain_one()
            for qb, m, kt in units:
                pt2, c0 = s_exp_unit(qb, m, kt)
                pipe.append((qb, m, kt, pt2, c0))
                if len(pipe) > 2:
                    drain_one()
            while pipe:
                drain_one()

            # tail: the carried pair plus last qb's phase C; the final
            # pair is stored as two singles on separate queues so the last
            # transfer is half-sized
            c_pair(2, 6, tail=True)
            for ot0 in (0, 2, 4):
                c_pair(3, ot0, tail=True)
            for k, ot in enumerate((6, 7)):
                ps = psC.tile([P, 512], F32, tag="pv", name=f"c3_{ot}")
                for mm in range(4):
                    nc.tensor.matmul(
                        ps,
                        lhsT=wo_sb[:, mm, ts(ot, P)],
                        rhs=HOP[:, mm, ds(3 * 512, 512)],
                        start=(mm == 0),
                        stop=(mm == 3),
                    )
                ob2 = obpool.tile([P, 2, 512], BF16, tag="ob")
                if k:
                    nc.scalar.copy(ob2[:, 0, :], ps)
                else:
                    nc.vector.tensor_copy(ob2[:, 0, :], ps)
                (nc.sync if k else nc.gpsimd).dma_start(
                    outT_t[:, ot : ot + 1, ds(3 * 512, 512)], ob2[:, 0:1, :]
                )

        ptile_cm.__exit__(None, None, None)


# ---------------- host side ----------------

def _host_tables():
    import ml_dtypes

    i = np.arange(32, dtype=np.float32)
    inv_freq = (THETA ** (2.0 * i / DK)).astype(np.float32)
    t = np.arange(S, dtype=np.float32)
    ang = t[:, None] / inv_freq[None, :]  # [S, 32]
    c = np.cos(ang).astype(np.float32).T  # [32, S]
    sn = np.sin(ang).astype(np.float32).T
    cosP = np.tile(c, (4, 1))  # [128, S]
    sinP = np.tile(sn, (4, 1))
    sign = np.repeat(np.array([-1.0, 1.0, -1.0, 1.0], dtype=np.float32), 32)
    sinP = sinP * sign[:, None]

    kk = np.arange(P)[:, None]
    qq = np.arange(P)[None, :]
    keep = (kk <= qq).astype(ml_dtypes.bfloat16)  # [128,128]
    trimask = np.tile(keep, (1, 2))  # [128, 2*128] (both head halves)
    bf = ml_dtypes.bfloat16
    return cosP.astype(bf), sinP.astype(bf), trimask


_PERM = np.concatenate(
    [np.concatenate([h * 64 + np.arange(0, 64, 2), h * 64 + np.arange(1, 64, 2)])
     for h in range(NH)]
)

_NC_CACHE = {}


def make_in_maps(x, Wq, Wk, Wv, Wo):
    import ml_dtypes

    bf = ml_dtypes.bfloat16
    cosP, sinP, trimask = _host_tables()
    in_maps = []
    for c in range(8):
        b, hh = c // 2, c % 2
        sl = slice(hh * HD, (hh + 1) * HD)
        in_maps.append(
            {
                "xT": np.ascontiguousarray(x[b].T).astype(bf),
                "wqT": np.ascontiguousarray(Wq[sl, :][_PERM].T).astype(bf),
                "wkT": np.ascontiguousarray(Wk[sl, :][_PERM].T).astype(bf),
                "wvT": np.ascontiguousarray(Wv[sl, :].T).astype(bf),
                "woT": np.ascontiguousarray(Wo[:, sl].T).astype(bf),
                "cosP": cosP,
                "sinP": sinP,
                "trimask": trimask,
                "onesc": np.ones((P, P), dtype=bf),
            }
        )
    return in_maps


def gather_out(core_outs):
    out = np.empty((B, S, D), dtype=np.float32)
    for b in range(B):
        a = np.asarray(core_outs[2 * b]["outT"], dtype=np.float32)
        bb = np.asarray(core_outs[2 * b + 1]["outT"], dtype=np.float32)
        out[b] = (a + bb).T
    return out


def kernel(x, Wq, Wk, Wv, Wo):
    x = np.asarray(x, dtype=np.float32)
    Wq = np.asarray(Wq, dtype=np.float32)
    Wk = np.asarray(Wk, dtype=np.float32)
    Wv = np.asarray(Wv, dtype=np.float32)
    Wo = np.asarray(Wo, dtype=np.float32)

    if "nc" not in _NC_CACHE:
        _NC_CACHE["nc"] = build_attention_nc()
    nc = _NC_CACHE["nc"]

    in_maps = make_in_maps(x, Wq, Wk, Wv, Wo)
    res = run_bass_kernel_spmd(nc, in_maps, core_ids=list(range(8)))
    return gather_out(res.results)
